# revision 12
# baseline (speedup 1.0000x reference)
"""Trainium2 Bass kernel for the pointer-generator decoder step (nn_Decoder).

Strategy (8 NeuronCores):
  - Phase 1 (LSTM + attention): data-parallel over batch. Core c owns batch
    rows [8c, 8c+8). Encoder tensors (the big per-batch traffic) are sharded
    by batch and shipped in bf16.
  - Phase 2 (vocab projection + softmax + scatter): tensor-parallel over the
    vocab axis. Wo2/bo2 (bf16) and the final distribution are sharded into 8
    column blocks of 6400 (padded 50500 -> 51200).
  - Cross-core glue: one small AllGather of (output, p_gen, attn) after
    phase 1, one tiny AllGather of the partial softmax denominators.
  - The pointer-copy scatter-add uses gpsimd local_scatter: per-partition
    (per-batch-row) scatter of attn values along the vocab axis with
    host-built int16 position maps, in 4 column blocks of 1600 per round
    (duplicate targets go to later rounds), summed into the distribution
    with DVE adds. No indirect DMA involved.

Precision: bf16 for the traffic/compute-heavy matmul paths (tanh/scores
inputs, c_t contraction, vocab projection); f16 for the scattered attn
values; fp32 for the LSTM, softmax statistics, and everything written out.

Self-contained: hardcodes all shapes from the problem spec.
"""

import ml_dtypes
import numpy as np

import concourse.bacc as bacc
import concourse.bass as bass
import concourse.mybir as mybir
import concourse.tile as tile
from concourse.bass_utils import run_bass_kernel_spmd
from concourse.masks import make_identity

F32 = mybir.dt.float32
BF16 = mybir.dt.bfloat16
F16 = mybir.dt.float16
I16 = mybir.dt.int16
I32 = mybir.dt.int32
NP_BF16 = ml_dtypes.bfloat16

NC = 8                      # cores
B, T, H, E, V, X = 64, 400, 256, 128, 50000, 500
BC = B // NC                # batch rows per core = 8
VFULL = V + X               # 50500
VC = 6400                   # vocab columns per core (8*6400 = 51200 >= 50500)
VPAD = VC * NC
H2 = 2 * H                  # 512
NEG_BIG = -200.0            # pad bias -> exp() == 0 in f32
NBLK = 4                    # local_scatter column blocks per shard
BLK = VC // NBLK            # 1600 (< 2048 gpsimd local limit)

# vocab matmul column tiling
VT_SIZES = [512] * 12 + [256]          # sum = 6400
assert sum(VT_SIZES) == VC


def _f32(x):
    return np.ascontiguousarray(np.asarray(x), dtype=np.float32)


def _bf16(x):
    return np.ascontiguousarray(np.asarray(x, dtype=np.float32).astype(NP_BF16))


def _i32(x):
    return np.ascontiguousarray(np.asarray(x), dtype=np.int32)


def _plan_scatter(ebv: np.ndarray):
    """Host-side plan for the pointer scatter-add via gpsimd local_scatter.

    Returns idx_maps[r][c] = int16 [B, NBLK, T]: for round r, core c, block k:
    idx_maps[r][c][b, k, t] = local position (0..BLK) of target ebv[b, t]
    within block k of core c's shard if that pair belongs to (c, k, r),
    else -1. Within one (b, c, k, r) all positions are unique.
    """
    ebv = np.asarray(ebv).astype(np.int64).reshape(B, T)
    core = ebv // VC
    jl = ebv - core * VC
    blk = jl // BLK
    pos = jl - blk * BLK

    # occurrence rank of each (b, target) pair
    occ = np.zeros((B, T), np.int64)
    for b in range(B):
        seen = {}
        row = ebv[b]
        for t in range(T):
            v = int(row[t])
            occ[b, t] = seen.get(v, 0)
            seen[v] = occ[b, t] + 1
    R = int(occ.max()) + 1

    idx_maps = []
    for r in range(R):
        per_core = []
        for c in range(NC):
            m = np.full((B, NBLK, T), -1, np.int16)
            sel = (core == c) & (occ == r)
            bb, tt = np.nonzero(sel)
            m[bb, blk[bb, tt], tt] = pos[bb, tt].astype(np.int16)
            per_core.append(m)
        idx_maps.append(per_core)
    return idx_maps


def build_program(n_rounds):
    """Build the SPMD Bass program (same on all cores)."""
    nc = bacc.Bacc("TRN2", target_bir_lowering=False, debug=False, num_devices=NC)

    # ---------------- I/O declarations ----------------
    def din(name, shape, dtype=F32):
        return nc.dram_tensor(name, list(shape), dtype, kind="ExternalInput")

    def dout(name, shape, dtype=F32):
        return nc.dram_tensor(name, list(shape), dtype, kind="ExternalOutput")

    y_idx = din("y_idx", [BC, 1], I32)
    emb = din("emb", [V, E])
    ct1T = din("ct1T", [H2, BC])          # c_t_1 transposed slice
    h0T = din("h0T", [H, BC])
    c0r = din("c0r", [BC, H])             # c0 rows slice
    efT = din("efT", [H2, BC * T], BF16)  # encoder_feature transposed slice
    enc = din("enc", [BC * T, H2], BF16)  # encoder_outputs slice
    stmt = din("stmt", [BC, T])
    maskin = din("maskin", [BC, T])
    wxcT = din("wxcT", [H2 + E, E])       # [640, 128]
    bxc = din("bxc", [E, 1])
    wihT = din("wihT", [E, 4 * H])        # [128, 1024]
    whhT = din("whhT", [H, 4 * H])        # [256, 1024]
    bgr = din("bgr", [1, 4 * H])          # (b_ih + b_hh) row
    wdpT = din("wdpT", [H2, H2])          # [512, 512]
    bdpr = din("bdpr", [1, H2])
    wvT = din("wvT", [H2, 1], BF16)       # [512, 1]
    wpgT = din("wpgT", [4 * H + E, 1])    # [1152, 1]
    wo1T = din("wo1T", [3 * H, H])        # [768, 256]
    bo1r = din("bo1r", [1, H])
    wo2T = din("wo2T", [E * 2, VC], BF16)  # [256, 6400] shard
    bo2r = din("bo2r", [1, VC], BF16)     # padded with NEG_BIG
    bpgs = din("bpgs", [BC, 1])           # p_gen bias (replicated column)
    scidx = [din(f"scidx{r}", [B, NBLK, T], I16) for r in range(n_rounds)]

    fin = dout("fin", [B, VC])            # final_dist shard
    hs_o = dout("hs_o", [BC, H])
    cs_o = dout("cs_o", [BC, H])
    ct_o = dout("ct_o", [BC, H2])
    at_o = dout("at_o", [BC, T])
    pg_o = dout("pg_o", [BC, 1])

    AGW = 672                             # allgather row width (256+1+400 padded)

    Sig = mybir.ActivationFunctionType.Sigmoid
    Tanh = mybir.ActivationFunctionType.Tanh
    Exp = mybir.ActivationFunctionType.Exp
    Ident = mybir.ActivationFunctionType.Identity

    with tile.TileContext(nc) as tc:
        with (
            tc.tile_pool(name="const", bufs=1) as cp,
            tc.tile_pool(name="work", bufs=1) as wp,
            tc.tile_pool(name="encp", bufs=2) as encp,
            tc.tile_pool(name="ps_t", bufs=2, space="PSUM") as ps_t,
            tc.tile_pool(name="ps_mm", bufs=2, space="PSUM") as ps_mm,
            tc.tile_pool(name="ps_row", bufs=2, space="PSUM") as ps_row,
            tc.tile_pool(name="ps_lg", bufs=2, space="PSUM") as ps_lg,
            tc.tile_pool(name="dram", bufs=1, space="DRAM") as dp,
        ):
            # ---------------- constants / weights to SBUF ----------------
            ident = cp.tile([128, 128], F32)
            make_identity(nc, ident[:])
            identb = cp.tile([128, 128], BF16)
            make_identity(nc, identb[:])
            ones18 = cp.tile([1, 8], F32)
            nc.gpsimd.memset(ones18[:], 1.0)
            ones1 = cp.tile([1, 64], BF16)
            nc.gpsimd.memset(ones1[:], 1.0)
            ones8 = cp.tile([8, 1], F32)
            nc.gpsimd.memset(ones8[:], 1.0)

            def loadt(name, shape, src_ap, dtype=F32):
                t = cp.tile(shape, dtype, name=name)
                nc.sync.dma_start(out=t[:], in_=src_ap)
                return t

            # big phase-2 weight on the ACT HWDGE ring so it streams in
            # parallel with the attention-phase loads on the SP ring
            wo2_t = cp.tile([128, 2, VC], BF16, name="wo2_t")
            nc.scalar.dma_start(out=wo2_t[:],
                                in_=wo2T[:, :].rearrange("(k p) v -> p k v", p=128))

            wxc_t = loadt("wxc_t", [128, 5, E],
                          wxcT[:, :].rearrange("(k p) m -> p k m", p=128))
            wih_t = loadt("wih_t", [128, 4 * H], wihT[:, :])
            whh_t = loadt("whh_t", [128, 2, 4 * H],
                          whhT[:, :].rearrange("(k p) m -> p k m", p=128))
            wdp_t = loadt("wdp_t", [128, 4, H2],
                          wdpT[:, :].rearrange("(k p) m -> p k m", p=128))
            wv_t = loadt("wv_t", [128, 4, 1],
                         wvT[:, :].rearrange("(k p) m -> p k m", p=128), dtype=BF16)
            wpg_t = loadt("wpg_t", [128, 9, 1],
                          wpgT[:, :].rearrange("(k p) m -> p k m", p=128))
            wo1_t = loadt("wo1_t", [128, 6, H],
                          wo1T[:, :].rearrange("(k p) m -> p k m", p=128))
            bgr_t = loadt("bgr_t", [1, 4 * H], bgr[:, :])
            bdpr_t = loadt("bdpr_t", [1, H2], bdpr[:, :])
            bo1r_t = loadt("bo1r_t", [1, H], bo1r[:, :])
            bxc_t = loadt("bxc_t", [E, 1], bxc[:, :])
            bpg_t = loadt("bpg_t", [BC, 1], bpgs[:, :])

            ct1_t = loadt("ct1_t", [128, 4, BC],
                          ct1T[:, :].rearrange("(k p) b -> p k b", p=128))
            h0_t = loadt("h0_t", [128, 2, BC],
                         h0T[:, :].rearrange("(k p) b -> p k b", p=128))
            c0r_t = loadt("c0r_t", [BC, H], c0r[:, :])
            stmt_t = loadt("stmt_t", [BC, T], stmt[:, :])
            mask_t = loadt("mask_t", [BC, T], maskin[:, :])

            def transpose_f32(name, src_ap, p_out, f_out):
                """[f_out, p_out] <- transpose of src_ap [p_out, f_out]."""
                pT = ps_t.tile([128, 128], F32, tag="t", name=f"{name}_ps")
                nc.tensor.transpose(pT[:f_out, :p_out], src_ap,
                                    ident[:p_out, :p_out])
                t = wp.tile([f_out, p_out], F32, name=name)
                nc.scalar.copy(t[:], pT[:f_out, :p_out])
                return t

            # ---------------- embedding gather + x projection ----------------
            yidx_t = loadt("yidx_t", [BC, 1], y_idx[:, :], dtype=I32)
            yemb = wp.tile([BC, E], F32, tag="yemb")
            nc.gpsimd.indirect_dma_start(
                out=yemb[:], out_offset=None, in_=emb[:, :],
                in_offset=bass.IndirectOffsetOnAxis(ap=yidx_t[:, :1], axis=0),
            )
            yembT = transpose_f32("yembT", yemb[:], BC, E)

            # xT = WxcT.T-chunks @ [ct1T; yembT] + bxc   -> [128, 8]
            x_ps = ps_mm.tile([E, BC], F32, tag="mm")
            for k in range(5):
                rhs = ct1_t[:, k, :] if k < 4 else yembT[:]
                nc.tensor.matmul(x_ps[:], wxc_t[:, k, :], rhs,
                                 start=(k == 0), stop=(k == 4))
            xT = wp.tile([E, BC], F32, tag="xT")
            nc.scalar.activation(xT[:], x_ps[:], Ident, bias=bxc_t[:, :1])

            # ---------------- LSTM gates (row layout [8, 1024]) ---------------
            # gates[b, :] = x @ Wih.T + h0 @ Whh.T + b; order i|f|g|o
            gate_rows = []
            for half in range(2):                      # [0,512) / [512,1024)
                sl = slice(half * 512, (half + 1) * 512)
                g_ps = ps_row.tile([BC, 512], F32, tag="row", name=f"g_ps{half}")
                nc.tensor.matmul(g_ps[:], ones18[:, :BC], bgr_t[:, sl],
                                 start=True, stop=False)
                nc.tensor.matmul(g_ps[:], xT[:], wih_t[:, sl],
                                 start=False, stop=False)
                for k in range(2):
                    nc.tensor.matmul(g_ps[:], h0_t[:, k, :], whh_t[:, k, sl],
                                     start=False, stop=(k == 1))
                gate_rows.append(g_ps)
            sig_if = wp.tile([BC, 512], F32, tag="sig_if")
            nc.scalar.activation(sig_if[:], gate_rows[0][:], Sig)
            tanh_g = wp.tile([BC, H], F32, tag="tanh_g")
            nc.scalar.activation(tanh_g[:], gate_rows[1][:, 0:H], Tanh)
            sig_o = wp.tile([BC, H], F32, tag="sig_o")
            nc.scalar.activation(sig_o[:], gate_rows[1][:, H:2 * H], Sig)

            m1 = wp.tile([BC, H], F32, tag="m1")
            nc.vector.tensor_mul(m1[:], sig_if[:, H:2 * H], c0r_t[:])
            m2 = wp.tile([BC, H], F32, tag="m2")
            nc.vector.tensor_mul(m2[:], sig_if[:, 0:H], tanh_g[:])
            cs_row = wp.tile([BC, H], F32, tag="cs_row")
            nc.vector.tensor_add(cs_row[:], m1[:], m2[:])
            tanh_cs = wp.tile([BC, H], F32, tag="tanh_cs")
            nc.scalar.activation(tanh_cs[:], cs_row[:], Tanh)
            hs_row = wp.tile([BC, H], F32, tag="hs_row")
            nc.vector.tensor_mul(hs_row[:], sig_o[:], tanh_cs[:])
            nc.sync.dma_start(out=hs_o[:, :], in_=hs_row[:])
            nc.sync.dma_start(out=cs_o[:, :], in_=cs_row[:])

            hsT = [transpose_f32(f"hsT{k}", hs_row[:, k * 128:(k + 1) * 128],
                                 BC, 128) for k in range(2)]
            csT = [transpose_f32(f"csT{k}", cs_row[:, k * 128:(k + 1) * 128],
                                 BC, 128) for k in range(2)]
            sthT = hsT + csT     # s_t_hat^T = [h_s; c_s] as 4 chunks of [128, 8]

            # ---------------- attention ----------------
            # dec_fea row [8, 512] then transpose to per-chunk bias columns
            d_ps = ps_row.tile([BC, H2], F32, tag="row", name="d_ps")
            nc.tensor.matmul(d_ps[:], ones18[:, :BC], bdpr_t[:, :],
                             start=True, stop=False)
            for k in range(4):
                nc.tensor.matmul(d_ps[:], sthT[k][:], wdp_t[:, k, :],
                                 start=False, stop=(k == 3))
            dec_row = wp.tile([BC, H2], F32, tag="dec_row")
            nc.scalar.copy(dec_row[:], d_ps[:])
            decT = [transpose_f32(f"decT{k}", dec_row[:, k * 128:(k + 1) * 128],
                                  BC, 128) for k in range(4)]

            # scores[b, t] accumulated over 4 n-chunks; per-b PSUM rows.
            # Engine APs must start at partition 0/32/64/96, so per-b rows are
            # written into a [1, 8*T] free-concat tile and reshaped via DRAM.
            esc_all = wp.tile([1, BC * T], F32, tag="esc_all")
            with tc.tile_pool(name="eft", bufs=4) as efp, \
                 tc.tile_pool(name="th", bufs=3) as thp:
                ef_ts = []
                for nci in range(4):
                    ef_t = efp.tile([128, BC * T], BF16, tag="ef", name=f"ef{nci}")
                    nc.sync.dma_start(out=ef_t[:],
                                      in_=efT[nci * 128:(nci + 1) * 128, :])
                    ef_ts.append(ef_t)
                for b in range(BC):
                    sc_ps = ps_row.tile([1, T], F32, tag="row", name=f"sc{b}")
                    for nci in range(4):
                        th = thp.tile([128, T], BF16, tag="th", name=f"th{b}_{nci}")
                        nc.scalar.activation(th[:], ef_ts[nci][:, b * T:(b + 1) * T],
                                             Tanh, bias=decT[nci][:, b:b + 1])
                        nc.tensor.matmul(sc_ps[:, :], wv_t[:, nci, :], th[:],
                                         start=(nci == 0), stop=(nci == 3))
                    # exp while still in PSUM; write segment b of esc_all
                    nc.scalar.activation(esc_all[:, b * T:(b + 1) * T],
                                         sc_ps[:, :], Exp)
            # reshape [1, B*T] -> [B, T] via DRAM bounce (SBUF->SBUF
            # partition-crossing reshape DMAs are not HW-reliable)
            esc_d = dp.tile([BC, T], F32, tag="esc_d", name="esc_d")
            nc.sync.dma_start(out=esc_d[:].flatten().unsqueeze(0), in_=esc_all[:1, :])
            esc = wp.tile([BC, T], F32, tag="esc")
            nc.sync.dma_start(out=esc[:, :], in_=esc_d[:])

            # softmax over t (no max-subtraction needed: |scores| < ~8)
            em = wp.tile([BC, T], F32, tag="em")
            nc.vector.tensor_mul(em[:], esc[:], mask_t[:])
            z1 = wp.tile([BC, 1], F32, tag="z1")
            nc.vector.tensor_reduce(z1[:], em[:], mybir.AxisListType.X,
                                    mybir.AluOpType.add)
            rz1 = wp.tile([BC, 1], F32, tag="rz1")
            nc.vector.reciprocal(rz1[:], z1[:])
            at0 = wp.tile([BC, T], F32, tag="at0")
            nc.vector.tensor_mul(at0[:], em[:], rz1[:].to_broadcast([BC, T]))
            sm = wp.tile([BC, T], F32, tag="sm")
            nc.vector.tensor_mul(sm[:], stmt_t[:], mask_t[:])
            attn = wp.tile([BC, T], F32, tag="attn")
            nc.vector.tensor_add(attn[:], at0[:], sm[:])
            nc.sync.dma_start(out=at_o[:, :], in_=attn[:])

            # attn^T chunks (bf16) for the c_t matmul
            attn_bf = wp.tile([BC, T], BF16, tag="attn_bf")
            nc.vector.tensor_copy(attn_bf[:], attn[:])
            attnT = []
            for tch in range(4):
                tsz = min(128, T - tch * 128)
                pT = ps_t.tile([128, BC], BF16, tag="t", name=f"attnT{tch}_ps")
                nc.tensor.transpose(pT[:tsz, :],
                                    attn_bf[:, tch * 128: tch * 128 + tsz],
                                    identb[:BC, :BC])
                aT = wp.tile([128, BC], BF16, tag=f"attnT{tch}")
                nc.vector.tensor_copy(aT[:tsz, :], pT[:tsz, :])
                attnT.append(aT)

            # c_t[b, :] = sum_t attn[b, t] * enc[b, t, :]
            ct_all = wp.tile([1, BC * H2], F32, tag="ct_all")
            for b in range(BC):
                e3 = encp.tile([128, 3, H2], BF16, tag="enc3", name=f"e3_{b}")
                nc.sync.dma_start(
                    out=e3[:],
                    in_=enc[b * T: b * T + 384, :].rearrange(
                        "(a p) f -> p a f", p=128),
                )
                e1 = encp.tile([16, H2], BF16, tag="enc1", name=f"e1_{b}")
                nc.sync.dma_start(out=e1[:], in_=enc[b * T + 384: b * T + 400, :])
                ct_ps = ps_row.tile([1, H2], F32, tag="row", name=f"ct{b}")
                for tch in range(4):
                    if tch < 3:
                        lhs = attnT[tch][:, b:b + 1]
                        rhs = e3[:, tch, :]
                    else:
                        lhs = attnT[tch][:16, b:b + 1]
                        rhs = e1[:, :]
                    nc.tensor.matmul(ct_ps[:, :], lhs, rhs,
                                     start=(tch == 0), stop=(tch == 3))
                nc.scalar.copy(ct_all[:, b * H2:(b + 1) * H2], ct_ps[:, :])
            ct_d = dp.tile([BC, H2], F32, tag="ct_d", name="ct_d")
            nc.sync.dma_start(out=ct_d[:].flatten().unsqueeze(0), in_=ct_all[:1, :])
            ct_row = wp.tile([BC, H2], F32, tag="ct_row")
            nc.sync.dma_start(out=ct_row[:, :], in_=ct_d[:])
            nc.sync.dma_start(out=ct_o[:, :], in_=ct_row[:])

            ctT = [transpose_f32(f"ctT{k}", ct_row[:, k * 128:(k + 1) * 128],
                                 BC, 128) for k in range(4)]

            # ---------------- p_gen (row layout -> [8, 1] directly) -----------
            pg_ps = ps_mm.tile([BC, 1], F32, tag="mm", name="pg_ps")
            pg_chunks = ctT + sthT + [xT]
            for k in range(9):
                nc.tensor.matmul(pg_ps[:], pg_chunks[k][:], wpg_t[:, k, :],
                                 start=(k == 0), stop=(k == 8))
            pg_col = wp.tile([BC, 1], F32, tag="pg_col")
            nc.scalar.activation(pg_col[:], pg_ps[:], Sig, bias=bpg_t[:, :1])
            nc.sync.dma_start(out=pg_o[:, :], in_=pg_col[:])

            # ---------------- output projection (row layout [8, 256]) ---------
            out_chunks = hsT + ctT        # [h_s; c_t] -> 6 chunks of 128
            o_ps = ps_row.tile([BC, H], F32, tag="row", name="o_ps")
            nc.tensor.matmul(o_ps[:], ones18[:, :BC], bo1r_t[:, :],
                             start=True, stop=False)
            for k in range(6):
                nc.tensor.matmul(o_ps[:], out_chunks[k][:], wo1_t[:, k, :],
                                 start=False, stop=(k == 5))
            out_row = wp.tile([BC, H], F32, tag="out_row")
            nc.scalar.copy(out_row[:], o_ps[:])

            # ---------------- AllGather #1 ----------------
            agin = wp.tile([BC, AGW], F32, tag="agin")
            nc.gpsimd.memset(agin[:], 0.0)
            nc.vector.tensor_copy(agin[:, 0:H], out_row[:])
            nc.vector.tensor_copy(agin[:, H:H + 1], pg_col[:])
            nc.vector.tensor_copy(agin[:, H + 1:H + 1 + T], attn[:])
            ag_in_d = dp.tile([BC, AGW], F32, tag="ag_in")
            ag_out_d = dp.tile([B, AGW], F32, tag="ag_out")
            nc.sync.dma_start(out=ag_in_d[:], in_=agin[:])
            nc.gpsimd.collective_compute(
                "AllGather", mybir.AluOpType.bypass,
                replica_groups=[list(range(NC))],
                ins=[ag_in_d[:].opt()], outs=[ag_out_d[:].opt()],
            )
            oa = wp.tile([B, H], F32, tag="oa")
            nc.sync.dma_start(out=oa[:], in_=ag_out_d[:, 0:H])
            pga = wp.tile([B, 1], F32, tag="pga")
            nc.sync.dma_start(out=pga[:], in_=ag_out_d[:, H:H + 1])
            ata = wp.tile([B, T], F32, tag="ata")
            nc.sync.dma_start(out=ata[:], in_=ag_out_d[:, H + 1:H + 1 + T])

            # ---------------- pointer scatter (gpsimd local_scatter) ----------
            om = wp.tile([B, 1], F32, tag="om")      # 1 - p_gen
            nc.vector.tensor_scalar(om[:], pga[:], -1.0, 1.0,
                                    mybir.AluOpType.mult, mybir.AluOpType.add)
            axp16 = wp.tile([B, T], F16, tag="axp16")
            nc.vector.tensor_mul(axp16[:], ata[:], om[:].to_broadcast([B, T]))

            sc_tiles = [[None] * NBLK for _ in range(n_rounds)]
            for r in range(n_rounds):
                idx_t = wp.tile([B, NBLK, T], I16, tag=f"scidx{r}")
                nc.sync.dma_start(out=idx_t[:], in_=scidx[r][:, :, :])
                for k in range(NBLK):
                    st = wp.tile([B, BLK], F16, tag=f"sct{r}_{k}")
                    nc.gpsimd.local_scatter(
                        out_ap=st[:], data_ap=axp16[:], idxs_ap=idx_t[:, k, :],
                        channels=B, num_elems=BLK, num_idxs=T,
                    )
                    sc_tiles[r][k] = st

            # ---------------- logits + exp (vocab shard) ----------------
            _xp_cm = tc.tile_pool(name="expp", bufs=1)
            xp = _xp_cm.__enter__()
            _bsp_cm = tc.tile_pool(name="bo2p", bufs=2)
            bsp = _bsp_cm.__enter__()

            oa_bf = wp.tile([B, H], BF16, tag="oa_bf")
            nc.vector.tensor_copy(oa_bf[:], oa[:])
            oaT = []
            for k in range(2):
                pT = ps_t.tile([128, B], BF16, tag="t", name=f"oaT{k}_ps")
                nc.tensor.transpose(pT[:], oa_bf[:, k * 128:(k + 1) * 128],
                                    identb[:B, :B])
                ot = wp.tile([128, B], BF16, tag=f"oaT{k}")
                nc.vector.tensor_copy(ot[:], pT[:])
                oaT.append(ot)

            expd = xp.tile([B, VC], F32, tag="expd")
            zcols = wp.tile([B, len(VT_SIZES)], F32, tag="zcols")
            off = 0
            for vt, nv in enumerate(VT_SIZES):
                l_ps = ps_lg.tile([B, 512], F32, tag="lg", name=f"lg{vt}")
                sl = slice(off, off + nv)
                bo2s = bsp.tile([1, 512], BF16, tag="bo2s", name=f"bo2s{vt}")
                nc.scalar.dma_start(out=bo2s[:, :nv], in_=bo2r[:, sl])
                nc.tensor.matmul(l_ps[:, :nv], ones1[:, :B], bo2s[:, :nv],
                                 start=True, stop=False)
                for k in range(2):
                    nc.tensor.matmul(l_ps[:, :nv], oaT[k][:], wo2_t[:, k, sl],
                                     start=False, stop=(k == 1))
                nc.scalar.activation(expd[:, sl], l_ps[:, :nv], Exp,
                                     accum_out=zcols[:, vt:vt + 1])
                off += nv

            zp = wp.tile([B, 1], F32, tag="zp")
            nc.vector.tensor_reduce(zp[:], zcols[:], mybir.AxisListType.X,
                                    mybir.AluOpType.add)

            # ---------------- AllGather #2 (partial denominators) -------------
            zrow_ps = ps_t.tile([1, B], F32, tag="t", name="zrow_ps")
            nc.tensor.transpose(zrow_ps[:], zp[:], ident[:B, :B])
            zrow = wp.tile([1, B], F32, tag="zrow")
            nc.scalar.copy(zrow[:], zrow_ps[:])
            z_in_d = dp.tile([1, B], F32, tag="z_in")
            z_out_d = dp.tile([NC, B], F32, tag="z_out")
            nc.sync.dma_start(out=z_in_d[:], in_=zrow[:])
            nc.gpsimd.collective_compute(
                "AllGather", mybir.AluOpType.bypass,
                replica_groups=[list(range(NC))],
                ins=[z_in_d[:].opt()], outs=[z_out_d[:].opt()],
            )
            zall = wp.tile([NC, B], F32, tag="zall")
            nc.sync.dma_start(out=zall[:], in_=z_out_d[:])
            zsum_ps = ps_t.tile([1, B], F32, tag="t", name="zsum_ps")
            nc.tensor.matmul(zsum_ps[:], ones8[:], zall[:], start=True, stop=True)
            zsum = wp.tile([1, B], F32, tag="zsum")
            nc.scalar.copy(zsum[:], zsum_ps[:])
            zcol_ps = ps_t.tile([B, 1], F32, tag="t", name="zcol_ps")
            nc.tensor.transpose(zcol_ps[:], zsum[:], ident[:1, :1])
            zcol = wp.tile([B, 1], F32, tag="zcol")
            nc.scalar.copy(zcol[:], zcol_ps[:])
            rz = wp.tile([B, 1], F32, tag="rz")
            nc.vector.reciprocal(rz[:], zcol[:])
            scl = wp.tile([B, 1], F32, tag="scl")
            nc.vector.tensor_mul(scl[:], pga[:], rz[:])

            # final = p_gen * softmax + scattered attn; write per block
            nc.vector.tensor_mul(expd[:], expd[:], scl[:].to_broadcast([B, VC]))
            for k in range(NBLK):
                blk_sl = slice(k * BLK, (k + 1) * BLK)
                acc = sc_tiles[0][k]
                for r in range(1, n_rounds):
                    a2 = wp.tile([B, BLK], F16, tag=f"scacc{r}_{k}")
                    nc.vector.tensor_add(a2[:], acc[:], sc_tiles[r][k][:])
                    acc = a2
                nc.vector.tensor_add(expd[:, blk_sl], expd[:, blk_sl], acc[:])
                nc.sync.dma_start(out=fin[:, blk_sl], in_=expd[:, blk_sl])

            _bsp_cm.__exit__(None, None, None)
            _xp_cm.__exit__(None, None, None)

    nc.compile()
    return nc


def _prep_inputs(inputs):
    """Host-side sharding / transposition. Returns (in_maps, n_rounds)."""
    emb = _f32(inputs["embedding"])
    y = _i32(np.asarray(inputs["y_t_1"]).reshape(B, 1))
    ct1T = _f32(np.asarray(inputs["c_t_1"]).T)                    # [512, 64]
    h0T = _f32(np.asarray(inputs["h0"]).T)                        # [256, 64]
    c0 = _f32(inputs["c0"])                                       # [64, 256]
    ef3 = _f32(inputs["encoder_feature"]).reshape(B, T, H2)
    enc3 = _f32(inputs["encoder_outputs"])                        # [64, 400, 512]
    stmt = _f32(inputs["stmt_feature"])
    mask = _f32(inputs["enc_padding_mask"])

    wxcT = _f32(np.asarray(inputs["Wxc"]).T)                      # [640, 128]
    bxc = _f32(np.asarray(inputs["bxc"]).reshape(E, 1))
    wihT = _f32(np.asarray(inputs["W_ih"]).T)                     # [128, 1024]
    whhT = _f32(np.asarray(inputs["W_hh"]).T)                     # [256, 1024]
    bgr = _f32((np.asarray(inputs["b_ih"]) +
                np.asarray(inputs["b_hh"])).reshape(1, 4 * H))
    wdpT = _f32(np.asarray(inputs["Wdp"]).T)                      # [512, 512]
    bdpr = _f32(np.asarray(inputs["bdp"]).reshape(1, H2))
    wvT = _bf16(np.asarray(inputs["Wv"]).T)                       # [512, 1]
    wpgT = _f32(np.asarray(inputs["Wpg"]).T)                      # [1152, 1]
    bpg = _f32(np.full((BC, 1), np.asarray(inputs["bpg"]).reshape(-1)[0]))
    wo1T = _f32(np.asarray(inputs["Wo1"]).T)                      # [768, 256]
    bo1r = _f32(np.asarray(inputs["bo1"]).reshape(1, H))

    wo2 = _f32(inputs["Wo2"])                                     # [50000, 256]
    bo2 = _f32(inputs["bo2"])                                     # [50000]
    wo2T_pad = np.zeros((2 * E, VPAD), np.float32)
    wo2T_pad[:, :V] = wo2.T
    wo2T_pad = wo2T_pad.astype(NP_BF16)
    bo2_pad = np.full((VPAD,), NEG_BIG, np.float32)
    bo2_pad[:V] = bo2
    bo2_pad = bo2_pad.astype(NP_BF16)

    idx_maps = _plan_scatter(inputs["enc_batch_extend_vocab"])
    n_rounds = len(idx_maps)

    in_maps = []
    for c in range(NC):
        rb = slice(c * BC, (c + 1) * BC)
        vb = slice(c * VC, (c + 1) * VC)
        m = {
            "y_idx": y[rb],
            "emb": emb,
            "ct1T": np.ascontiguousarray(ct1T[:, rb]),
            "h0T": np.ascontiguousarray(h0T[:, rb]),
            "c0r": np.ascontiguousarray(c0[rb]),
            "efT": _bf16(ef3[rb].reshape(BC * T, H2).T),          # [512, 3200]
            "enc": _bf16(enc3[rb].reshape(BC * T, H2)),
            "stmt": np.ascontiguousarray(stmt[rb]),
            "maskin": np.ascontiguousarray(mask[rb]),
            "wxcT": wxcT, "bxc": bxc, "wihT": wihT, "whhT": whhT,
            "bgr": bgr, "wdpT": wdpT, "bdpr": bdpr, "wvT": wvT,
            "wpgT": wpgT, "bpgs": bpg, "wo1T": wo1T, "bo1r": bo1r,
            "wo2T": np.ascontiguousarray(wo2T_pad[:, vb]),
            "bo2r": np.ascontiguousarray(bo2_pad[vb].reshape(1, VC)),
        }
        for r in range(n_rounds):
            m[f"scidx{r}"] = np.ascontiguousarray(idx_maps[r][c])
        in_maps.append(m)
    return in_maps, n_rounds


def run(inputs, trace=False):
    """Build + run; returns (outputs_tuple, BassKernelResults)."""
    in_maps, n_rounds = _prep_inputs(inputs)
    nc = build_program(n_rounds)
    res = run_bass_kernel_spmd(
        nc, in_maps, core_ids=list(range(NC)), trace=trace,
        trace_cores=[0] if trace else None,
    )
    rs = res.results

    final = np.concatenate([rs[c]["fin"] for c in range(NC)], axis=1)[:, :VFULL]
    h_s = np.concatenate([rs[c]["hs_o"] for c in range(NC)], axis=0)
    c_s = np.concatenate([rs[c]["cs_o"] for c in range(NC)], axis=0)
    c_t = np.concatenate([rs[c]["ct_o"] for c in range(NC)], axis=0)
    attn = np.concatenate([rs[c]["at_o"] for c in range(NC)], axis=0)
    p_gen = np.concatenate([rs[c]["pg_o"] for c in range(NC)], axis=0)
    coverage = _f32(inputs["coverage"])
    outs = (final, h_s, c_s, c_t, attn, p_gen, coverage)
    return outs, res


def kernel(**inputs):
    outs, _ = run(inputs, trace=False)
    return outs


# revision 16
# speedup vs baseline: 1.0350x; 1.0350x over previous
"""Trainium2 Bass kernel for the pointer-generator decoder step (nn_Decoder).

Strategy (8 NeuronCores):
  - Phase 1 (LSTM + attention): data-parallel over batch. Core c owns batch
    rows [8c, 8c+8). Encoder tensors (the big per-batch traffic) are sharded
    by batch and shipped in bf16.
  - Phase 2 (vocab projection + softmax + scatter): tensor-parallel over the
    vocab axis. Wo2/bo2 (bf16) and the final distribution are sharded into 8
    column blocks of 6400 (padded 50500 -> 51200).
  - Cross-core glue: one small AllGather of (output, p_gen, attn) after
    phase 1, one tiny AllGather of the partial softmax denominators.
  - The pointer-copy scatter-add uses gpsimd local_scatter: per-partition
    (per-batch-row) scatter of attn values along the vocab axis with
    host-built int16 position maps, in 4 column blocks of 1600 per round
    (duplicate targets go to later rounds), summed into the distribution
    with DVE adds. No indirect DMA involved.

Precision: bf16 for the traffic/compute-heavy matmul paths (tanh/scores
inputs, c_t contraction, vocab projection); f16 for the scattered attn
values; fp32 for the LSTM, softmax statistics, and everything written out.

Self-contained: hardcodes all shapes from the problem spec.
"""

import ml_dtypes
import numpy as np

import concourse.bacc as bacc
import concourse.bass as bass
import concourse.mybir as mybir
import concourse.tile as tile
from concourse.bass_utils import run_bass_kernel_spmd
from concourse.masks import make_identity

F32 = mybir.dt.float32
BF16 = mybir.dt.bfloat16
F16 = mybir.dt.float16
I16 = mybir.dt.int16
I32 = mybir.dt.int32
NP_BF16 = ml_dtypes.bfloat16

NC = 8                      # cores
B, T, H, E, V, X = 64, 400, 256, 128, 50000, 500
BC = B // NC                # batch rows per core = 8
VFULL = V + X               # 50500
VC = 6400                   # vocab columns per core (8*6400 = 51200 >= 50500)
VPAD = VC * NC
H2 = 2 * H                  # 512
NEG_BIG = -200.0            # pad bias -> exp() == 0 in f32
NBLK = 4                    # local_scatter column blocks per shard
BLK = VC // NBLK            # 1600 (< 2048 gpsimd local limit)

# vocab matmul column tiling
VT_SIZES = [512] * 12 + [256]          # sum = 6400
assert sum(VT_SIZES) == VC


def _f32(x):
    return np.ascontiguousarray(np.asarray(x), dtype=np.float32)


def _bf16(x):
    return np.ascontiguousarray(np.asarray(x, dtype=np.float32).astype(NP_BF16))


def _i32(x):
    return np.ascontiguousarray(np.asarray(x), dtype=np.int32)


def _plan_scatter(ebv: np.ndarray):
    """Host-side plan for the pointer scatter-add via gpsimd local_scatter.

    Returns idx_maps[r][c] = int16 [B, NBLK, T]: for round r, core c, block k:
    idx_maps[r][c][b, k, t] = local position (0..BLK) of target ebv[b, t]
    within block k of core c's shard if that pair belongs to (c, k, r),
    else -1. Within one (b, c, k, r) all positions are unique.
    """
    ebv = np.asarray(ebv).astype(np.int64).reshape(B, T)
    core = ebv // VC
    jl = ebv - core * VC
    blk = jl // BLK
    pos = jl - blk * BLK

    # occurrence rank of each (b, target) pair
    occ = np.zeros((B, T), np.int64)
    for b in range(B):
        seen = {}
        row = ebv[b]
        for t in range(T):
            v = int(row[t])
            occ[b, t] = seen.get(v, 0)
            seen[v] = occ[b, t] + 1
    R = int(occ.max()) + 1

    idx_maps = []
    for r in range(R):
        per_core = []
        for c in range(NC):
            m = np.full((B, NBLK, T), -1, np.int16)
            sel = (core == c) & (occ == r)
            bb, tt = np.nonzero(sel)
            m[bb, blk[bb, tt], tt] = pos[bb, tt].astype(np.int16)
            per_core.append(m)
        idx_maps.append(per_core)
    return idx_maps


def build_program(n_rounds):
    """Build the SPMD Bass program (same on all cores)."""
    nc = bacc.Bacc("TRN2", target_bir_lowering=False, debug=False, num_devices=NC)

    # ---------------- I/O declarations ----------------
    def din(name, shape, dtype=F32):
        return nc.dram_tensor(name, list(shape), dtype, kind="ExternalInput")

    def dout(name, shape, dtype=F32):
        return nc.dram_tensor(name, list(shape), dtype, kind="ExternalOutput")

    y_idx = din("y_idx", [BC, 1], I32)
    emb = din("emb", [V, E])
    ct1T = din("ct1T", [H2, BC])          # c_t_1 transposed slice
    h0T = din("h0T", [H, BC])
    c0r = din("c0r", [BC, H])             # c0 rows slice
    efT = din("efT", [H2, BC * T], BF16)  # encoder_feature transposed slice
    enc = din("enc", [BC * 512, H2], BF16)  # encoder_outputs slice (t padded)
    stmt = din("stmt", [BC, T])
    maskin = din("maskin", [BC, T])
    wxcT = din("wxcT", [H2 + E, E])       # [640, 128]
    bxc = din("bxc", [E, 1])
    wihT = din("wihT", [E, 4 * H])        # [128, 1024]
    whhT = din("whhT", [H, 4 * H])        # [256, 1024]
    bgr = din("bgr", [1, 4 * H])          # (b_ih + b_hh) row
    wdpT = din("wdpT", [H2, H2], BF16)    # [512, 512]
    bdpr = din("bdpr", [1, H2], BF16)
    wvT = din("wvT", [H2, 1], BF16)       # [512, 1]
    wpgT = din("wpgT", [4 * H + E, 1], BF16)  # [1152, 1]
    wo1T = din("wo1T", [3 * H, H], BF16)  # [768, 256]
    bo1r = din("bo1r", [1, H], BF16)
    wo2T = din("wo2T", [E * 2, VC], BF16)  # [256, 6400] shard
    bo2r = din("bo2r", [1, VC], BF16)     # padded with NEG_BIG
    bpgs = din("bpgs", [BC, 1])           # p_gen bias (replicated column)
    scidx = [din(f"scidx{r}", [B, NBLK, T], I16) for r in range(n_rounds)]

    fin = dout("fin", [B, VC])            # final_dist shard
    hs_o = dout("hs_o", [BC, H])
    cs_o = dout("cs_o", [BC, H])
    ct_o = dout("ct_o", [BC, H2])
    at_o = dout("at_o", [BC, T])
    pg_o = dout("pg_o", [BC, 1])

    AGW = 672                             # allgather row width (256+1+400 padded)

    Sig = mybir.ActivationFunctionType.Sigmoid
    Tanh = mybir.ActivationFunctionType.Tanh
    Exp = mybir.ActivationFunctionType.Exp
    Ident = mybir.ActivationFunctionType.Identity

    with tile.TileContext(nc) as tc:
        with (
            tc.tile_pool(name="const", bufs=1) as cp,
            tc.tile_pool(name="work", bufs=1) as wp,
            tc.tile_pool(name="encp", bufs=2) as encp,
            tc.tile_pool(name="ps_t", bufs=2, space="PSUM") as ps_t,
            tc.tile_pool(name="ps_mm", bufs=2, space="PSUM") as ps_mm,
            tc.tile_pool(name="ps_row", bufs=2, space="PSUM") as ps_row,
            tc.tile_pool(name="ps_lg", bufs=2, space="PSUM") as ps_lg,
            tc.tile_pool(name="dram", bufs=1, space="DRAM") as dp,
        ):
            # ---------------- constants / weights to SBUF ----------------
            ident = cp.tile([128, 128], F32)
            make_identity(nc, ident[:])
            identb = cp.tile([128, 128], BF16)
            make_identity(nc, identb[:])
            ones18 = cp.tile([1, 8], F32)
            nc.gpsimd.memset(ones18[:], 1.0)
            ones18b = cp.tile([1, 8], BF16)
            nc.gpsimd.memset(ones18b[:], 1.0)
            ones1 = cp.tile([1, 64], BF16)
            nc.gpsimd.memset(ones1[:], 1.0)
            ones8 = cp.tile([8, 1], F32)
            nc.gpsimd.memset(ones8[:], 1.0)

            def loadt(name, shape, src_ap, dtype=F32):
                t = cp.tile(shape, dtype, name=name)
                nc.sync.dma_start(out=t[:], in_=src_ap)
                return t

            # big phase-2 weight on the ACT HWDGE ring so it streams in
            # parallel with the attention-phase loads on the SP ring
            wo2_t = cp.tile([128, 2, VC], BF16, name="wo2_t")
            nc.scalar.dma_start(out=wo2_t[:],
                                in_=wo2T[:, :].rearrange("(k p) v -> p k v", p=128))

            wxc_t = loadt("wxc_t", [128, 5, E],
                          wxcT[:, :].rearrange("(k p) m -> p k m", p=128))
            wih_t = loadt("wih_t", [128, 4 * H], wihT[:, :])
            whh_t = loadt("whh_t", [128, 2, 4 * H],
                          whhT[:, :].rearrange("(k p) m -> p k m", p=128))
            wdp_t = loadt("wdp_t", [128, 4, H2],
                          wdpT[:, :].rearrange("(k p) m -> p k m", p=128),
                          dtype=BF16)
            wv_t = loadt("wv_t", [128, 4, 1],
                         wvT[:, :].rearrange("(k p) m -> p k m", p=128), dtype=BF16)
            wpg_t = loadt("wpg_t", [128, 9, 1],
                          wpgT[:, :].rearrange("(k p) m -> p k m", p=128),
                          dtype=BF16)
            wo1_t = loadt("wo1_t", [128, 6, H],
                          wo1T[:, :].rearrange("(k p) m -> p k m", p=128),
                          dtype=BF16)
            bgr_t = loadt("bgr_t", [1, 4 * H], bgr[:, :])
            bdpr_t = loadt("bdpr_t", [1, H2], bdpr[:, :], dtype=BF16)
            bo1r_t = loadt("bo1r_t", [1, H], bo1r[:, :], dtype=BF16)
            bxc_t = loadt("bxc_t", [E, 1], bxc[:, :])
            bpg_t = loadt("bpg_t", [BC, 1], bpgs[:, :])

            ct1_t = loadt("ct1_t", [128, 4, BC],
                          ct1T[:, :].rearrange("(k p) b -> p k b", p=128))
            h0_t = loadt("h0_t", [128, 2, BC],
                         h0T[:, :].rearrange("(k p) b -> p k b", p=128))
            c0r_t = loadt("c0r_t", [BC, H], c0r[:, :])
            stmt_t = loadt("stmt_t", [BC, T], stmt[:, :])
            mask_t = loadt("mask_t", [BC, T], maskin[:, :])

            def transpose_f32(name, src_ap, p_out, f_out):
                """[f_out, p_out] <- transpose of src_ap [p_out, f_out]."""
                pT = ps_t.tile([128, 128], F32, tag="t", name=f"{name}_ps")
                nc.tensor.transpose(pT[:f_out, :p_out], src_ap,
                                    ident[:p_out, :p_out])
                t = wp.tile([f_out, p_out], F32, name=name)
                nc.scalar.copy(t[:], pT[:f_out, :p_out])
                return t

            def transpose_bf(name, src_bf_ap, p_out, f_out):
                """bf16 [f_out, p_out] <- transpose of bf16 src [p_out, f_out]."""
                pT = ps_t.tile([128, 128], BF16, tag="t", name=f"{name}_ps")
                nc.tensor.transpose(pT[:f_out, :p_out], src_bf_ap,
                                    identb[:p_out, :p_out])
                t = wp.tile([f_out, p_out], BF16, name=name)
                nc.vector.tensor_copy(t[:], pT[:f_out, :p_out])
                return t

            # ---------------- embedding gather + x projection ----------------
            yidx_t = loadt("yidx_t", [BC, 1], y_idx[:, :], dtype=I32)
            yemb = wp.tile([BC, E], F32, tag="yemb")
            nc.gpsimd.indirect_dma_start(
                out=yemb[:], out_offset=None, in_=emb[:, :],
                in_offset=bass.IndirectOffsetOnAxis(ap=yidx_t[:, :1], axis=0),
            )
            yembT = transpose_f32("yembT", yemb[:], BC, E)

            # xT = WxcT.T-chunks @ [ct1T; yembT] + bxc   -> [128, 8]
            x_ps = ps_mm.tile([E, BC], F32, tag="mm")
            for k in range(5):
                rhs = ct1_t[:, k, :] if k < 4 else yembT[:]
                nc.tensor.matmul(x_ps[:], wxc_t[:, k, :], rhs,
                                 start=(k == 0), stop=(k == 4))
            xT = wp.tile([E, BC], F32, tag="xT")
            nc.scalar.activation(xT[:], x_ps[:], Ident, bias=bxc_t[:, :1])

            # ---------------- LSTM gates (row layout [8, 1024]) ---------------
            # gates[b, :] = x @ Wih.T + h0 @ Whh.T + b; order i|f|g|o
            gate_rows = []
            for half in range(2):                      # [0,512) / [512,1024)
                sl = slice(half * 512, (half + 1) * 512)
                g_ps = ps_row.tile([BC, 512], F32, tag="row", name=f"g_ps{half}")
                nc.tensor.matmul(g_ps[:], ones18[:, :BC], bgr_t[:, sl],
                                 start=True, stop=False)
                nc.tensor.matmul(g_ps[:], xT[:], wih_t[:, sl],
                                 start=False, stop=False)
                for k in range(2):
                    nc.tensor.matmul(g_ps[:], h0_t[:, k, :], whh_t[:, k, sl],
                                     start=False, stop=(k == 1))
                gate_rows.append(g_ps)
            sig_if = wp.tile([BC, 512], F32, tag="sig_if")
            nc.scalar.activation(sig_if[:], gate_rows[0][:], Sig)
            tanh_g = wp.tile([BC, H], F32, tag="tanh_g")
            nc.scalar.activation(tanh_g[:], gate_rows[1][:, 0:H], Tanh)
            sig_o = wp.tile([BC, H], F32, tag="sig_o")
            nc.scalar.activation(sig_o[:], gate_rows[1][:, H:2 * H], Sig)

            m1 = wp.tile([BC, H], F32, tag="m1")
            nc.vector.tensor_mul(m1[:], sig_if[:, H:2 * H], c0r_t[:])
            m2 = wp.tile([BC, H], F32, tag="m2")
            nc.vector.tensor_mul(m2[:], sig_if[:, 0:H], tanh_g[:])
            cs_row = wp.tile([BC, H], F32, tag="cs_row")
            nc.vector.tensor_add(cs_row[:], m1[:], m2[:])
            tanh_cs = wp.tile([BC, H], F32, tag="tanh_cs")
            nc.scalar.activation(tanh_cs[:], cs_row[:], Tanh)
            hs_row = wp.tile([BC, H], F32, tag="hs_row")
            nc.vector.tensor_mul(hs_row[:], sig_o[:], tanh_cs[:])
            nc.sync.dma_start(out=hs_o[:, :], in_=hs_row[:])
            nc.sync.dma_start(out=cs_o[:, :], in_=cs_row[:])

            hs_bf = wp.tile([BC, H], BF16, tag="hs_bf")
            nc.vector.tensor_copy(hs_bf[:], hs_row[:])
            cs_bf = wp.tile([BC, H], BF16, tag="cs_bf")
            nc.vector.tensor_copy(cs_bf[:], cs_row[:])
            hsT = [transpose_bf(f"hsT{k}", hs_bf[:, k * 128:(k + 1) * 128],
                                BC, 128) for k in range(2)]
            csT = [transpose_bf(f"csT{k}", cs_bf[:, k * 128:(k + 1) * 128],
                                BC, 128) for k in range(2)]
            sthT = hsT + csT     # s_t_hat^T = [h_s; c_s] as 4 chunks of [128, 8]

            # ---------------- attention ----------------
            # dec_fea row [8, 512] then transpose to per-chunk bias columns
            d_ps = ps_row.tile([BC, H2], F32, tag="row", name="d_ps")
            nc.tensor.matmul(d_ps[:], ones18b[:, :BC], bdpr_t[:, :],
                             start=True, stop=False)
            for k in range(4):
                nc.tensor.matmul(d_ps[:], sthT[k][:], wdp_t[:, k, :],
                                 start=False, stop=(k == 3))
            dec_row = wp.tile([BC, H2], BF16, tag="dec_row")
            nc.vector.tensor_copy(dec_row[:], d_ps[:])
            decT = [transpose_bf(f"decT{k}", dec_row[:, k * 128:(k + 1) * 128],
                                 BC, 128) for k in range(4)]

            # scores[b, t] accumulated over 4 n-chunks; per-b PSUM rows.
            # Engine APs must start at partition 0/32/64/96, so per-b rows are
            # written into a [1, 8*T] free-concat tile and reshaped via DRAM.
            esc_all = wp.tile([1, BC * T], F32, tag="esc_all")
            with tc.tile_pool(name="eft", bufs=2) as efp, \
                 tc.tile_pool(name="th", bufs=4) as thp:
                th_ts = []
                for nci in range(4):
                    ef_t = efp.tile([128, BC * T], BF16, tag="ef", name=f"ef{nci}")
                    nc.sync.dma_start(out=ef_t[:],
                                      in_=efT[nci * 128:(nci + 1) * 128, :])
                    # ef + dec_fea (free-broadcast per b), then one big tanh
                    ta = thp.tile([128, BC, T], BF16, tag="th", name=f"ta{nci}")
                    nc.vector.tensor_add(
                        ta[:],
                        ef_t[:].rearrange("p (b t) -> p b t", b=BC),
                        decT[nci][:].unsqueeze(2).to_broadcast([128, BC, T]))
                    th = ta[:].rearrange("p b t -> p (b t)")
                    nc.scalar.activation(th, th, Tanh)
                    th_ts.append(th)
                for b in range(BC):
                    sc_ps = ps_row.tile([1, T], F32, tag="row", name=f"sc{b}")
                    for nci in range(4):
                        nc.tensor.matmul(sc_ps[:, :], wv_t[:, nci, :],
                                         th_ts[nci][:, b * T:(b + 1) * T],
                                         start=(nci == 0), stop=(nci == 3))
                    # exp while still in PSUM; write segment b of esc_all
                    nc.scalar.activation(esc_all[:, b * T:(b + 1) * T],
                                         sc_ps[:, :], Exp)
            # reshape [1, B*T] -> [B, T] via DRAM bounce (SBUF->SBUF
            # partition-crossing reshape DMAs are not HW-reliable)
            esc_d = dp.tile([BC, T], F32, tag="esc_d", name="esc_d")
            nc.sync.dma_start(out=esc_d[:].flatten().unsqueeze(0), in_=esc_all[:1, :])
            esc = wp.tile([BC, T], F32, tag="esc")
            nc.sync.dma_start(out=esc[:, :], in_=esc_d[:])

            # softmax over t (no max-subtraction needed: |scores| < ~8)
            nc.vector.tensor_mul(esc[:], esc[:], mask_t[:])
            z1 = wp.tile([BC, 1], F32, tag="z1")
            nc.vector.tensor_reduce(z1[:], esc[:], mybir.AxisListType.X,
                                    mybir.AluOpType.add)
            rz1 = wp.tile([BC, 1], F32, tag="rz1")
            nc.vector.reciprocal(rz1[:], z1[:])
            sm = wp.tile([BC, T], F32, tag="sm")
            nc.vector.tensor_mul(sm[:], stmt_t[:], mask_t[:])
            attn = wp.tile([BC, T], F32, tag="attn")
            nc.vector.scalar_tensor_tensor(
                out=attn[:], in0=esc[:], scalar=rz1[:, :1], in1=sm[:],
                op0=mybir.AluOpType.mult, op1=mybir.AluOpType.add)
            nc.sync.dma_start(out=at_o[:, :], in_=attn[:])

            # attn^T chunks (bf16, zero-padded to 512) for the c_t matmul
            attn_bf = wp.tile([BC, 512], BF16, tag="attn_bf")
            nc.gpsimd.memset(attn_bf[:], 0.0)
            nc.vector.tensor_copy(attn_bf[:, :T], attn[:])
            attnT = []
            for tch in range(4):
                aT = transpose_bf(f"attnT{tch}",
                                  attn_bf[:, tch * 128:(tch + 1) * 128], BC, 128)
                attnT.append(aT)

            # c_t[b, :] = sum_t attn[b, t] * enc[b, t, :]
            ct_all = wp.tile([1, BC * H2], F32, tag="ct_all")
            for b in range(BC):
                e4 = encp.tile([128, 4, H2], BF16, tag="enc4", name=f"e4_{b}")
                nc.sync.dma_start(
                    out=e4[:],
                    in_=enc[b * 512:(b + 1) * 512, :].rearrange(
                        "(a p) f -> p a f", p=128),
                )
                ct_ps = ps_row.tile([1, H2], F32, tag="row", name=f"ct{b}")
                for tch in range(4):
                    nc.tensor.matmul(ct_ps[:, :], attnT[tch][:, b:b + 1],
                                     e4[:, tch, :],
                                     start=(tch == 0), stop=(tch == 3))
                nc.vector.tensor_copy(ct_all[:, b * H2:(b + 1) * H2], ct_ps[:, :])
            ct_d = dp.tile([BC, H2], F32, tag="ct_d", name="ct_d")
            nc.sync.dma_start(out=ct_d[:].flatten().unsqueeze(0), in_=ct_all[:1, :])
            ct_row = wp.tile([BC, H2], F32, tag="ct_row")
            nc.sync.dma_start(out=ct_row[:, :], in_=ct_d[:])
            nc.sync.dma_start(out=ct_o[:, :], in_=ct_row[:])

            ct_bf = wp.tile([BC, H2], BF16, tag="ct_bf")
            nc.vector.tensor_copy(ct_bf[:], ct_row[:])
            ctT = [transpose_bf(f"ctT{k}", ct_bf[:, k * 128:(k + 1) * 128],
                                BC, 128) for k in range(4)]
            xT_bf = wp.tile([E, BC], BF16, tag="xT_bf")
            nc.vector.tensor_copy(xT_bf[:], xT[:])

            # ---------------- p_gen (row layout -> [8, 1] directly) -----------
            pg_ps = ps_mm.tile([BC, 1], F32, tag="mm", name="pg_ps")
            pg_chunks = ctT + sthT + [xT_bf]
            for k in range(9):
                nc.tensor.matmul(pg_ps[:], pg_chunks[k][:], wpg_t[:, k, :],
                                 start=(k == 0), stop=(k == 8))
            pg_col = wp.tile([BC, 1], F32, tag="pg_col")
            nc.scalar.activation(pg_col[:], pg_ps[:], Sig, bias=bpg_t[:, :1])
            nc.sync.dma_start(out=pg_o[:, :], in_=pg_col[:])

            # ---------------- output projection (row layout [8, 256]) ---------
            out_chunks = hsT + ctT        # [h_s; c_t] -> 6 chunks of 128
            o_ps = ps_row.tile([BC, H], F32, tag="row", name="o_ps")
            nc.tensor.matmul(o_ps[:], ones18b[:, :BC], bo1r_t[:, :],
                             start=True, stop=False)
            for k in range(6):
                nc.tensor.matmul(o_ps[:], out_chunks[k][:], wo1_t[:, k, :],
                                 start=False, stop=(k == 5))
            out_row = wp.tile([BC, H], F32, tag="out_row")
            nc.scalar.copy(out_row[:], o_ps[:])

            # ---------------- AllGather #1 ----------------
            agin = wp.tile([BC, AGW], F32, tag="agin")
            nc.gpsimd.memset(agin[:], 0.0)
            nc.vector.tensor_copy(agin[:, 0:H], out_row[:])
            nc.vector.tensor_copy(agin[:, H:H + 1], pg_col[:])
            nc.vector.tensor_copy(agin[:, H + 1:H + 1 + T], attn[:])
            ag_in_d = dp.tile([BC, AGW], F32, tag="ag_in")
            ag_out_d = dp.tile([B, AGW], F32, tag="ag_out")
            nc.sync.dma_start(out=ag_in_d[:], in_=agin[:])
            nc.gpsimd.collective_compute(
                "AllGather", mybir.AluOpType.bypass,
                replica_groups=[list(range(NC))],
                ins=[ag_in_d[:].opt()], outs=[ag_out_d[:].opt()],
            )
            aga = wp.tile([B, AGW], F32, tag="aga")
            nc.sync.dma_start(out=aga[:], in_=ag_out_d[:, :])
            oa = aga[:, 0:H]
            pga = aga[:, H:H + 1]
            ata = aga[:, H + 1:H + 1 + T]

            # ---------------- pointer scatter (gpsimd local_scatter) ----------
            om = wp.tile([B, 1], F32, tag="om")      # 1 - p_gen
            nc.vector.tensor_scalar(om[:], pga, -1.0, 1.0,
                                    mybir.AluOpType.mult, mybir.AluOpType.add)
            axp16 = wp.tile([B, T], BF16, tag="axp16")
            nc.vector.tensor_mul(axp16[:], ata, om[:].to_broadcast([B, T]))

            sc_tiles = [[None] * NBLK for _ in range(n_rounds)]
            for r in range(n_rounds):
                idx_t = wp.tile([B, NBLK, T], I16, tag=f"scidx{r}")
                nc.sync.dma_start(out=idx_t[:], in_=scidx[r][:, :, :])
                for k in range(NBLK):
                    st = wp.tile([B, BLK], BF16, tag=f"sct{r}_{k}")
                    nc.gpsimd.local_scatter(
                        out_ap=st[:], data_ap=axp16[:], idxs_ap=idx_t[:, k, :],
                        channels=B, num_elems=BLK, num_idxs=T,
                    )
                    sc_tiles[r][k] = st

            # ---------------- logits + exp (vocab shard) ----------------
            _xp_cm = tc.tile_pool(name="expp", bufs=1)
            xp = _xp_cm.__enter__()
            _bsp_cm = tc.tile_pool(name="bo2p", bufs=2)
            bsp = _bsp_cm.__enter__()

            oa_bf = wp.tile([B, H], BF16, tag="oa_bf")
            nc.vector.tensor_copy(oa_bf[:], oa)
            oaT = []
            for k in range(2):
                pT = ps_t.tile([128, B], BF16, tag="t", name=f"oaT{k}_ps")
                nc.tensor.transpose(pT[:], oa_bf[:, k * 128:(k + 1) * 128],
                                    identb[:B, :B])
                ot = wp.tile([128, B], BF16, tag=f"oaT{k}")
                nc.vector.tensor_copy(ot[:], pT[:])
                oaT.append(ot)

            expd = xp.tile([B, VC], BF16, tag="expd")
            zcols = wp.tile([B, len(VT_SIZES)], F32, tag="zcols")
            off = 0
            for vt, nv in enumerate(VT_SIZES):
                l_ps = ps_lg.tile([B, 512], F32, tag="lg", name=f"lg{vt}")
                sl = slice(off, off + nv)
                bo2s = bsp.tile([1, 512], BF16, tag="bo2s", name=f"bo2s{vt}")
                nc.scalar.dma_start(out=bo2s[:, :nv], in_=bo2r[:, sl])
                nc.tensor.matmul(l_ps[:, :nv], ones1[:, :B], bo2s[:, :nv],
                                 start=True, stop=False)
                for k in range(2):
                    nc.tensor.matmul(l_ps[:, :nv], oaT[k][:], wo2_t[:, k, sl],
                                     start=False, stop=(k == 1))
                nc.scalar.activation(expd[:, sl], l_ps[:, :nv], Exp,
                                     accum_out=zcols[:, vt:vt + 1])
                off += nv

            zp = wp.tile([B, 1], F32, tag="zp")
            nc.vector.tensor_reduce(zp[:], zcols[:], mybir.AxisListType.X,
                                    mybir.AluOpType.add)

            # ---------------- AllGather #2 (partial denominators) -------------
            zrow_ps = ps_t.tile([1, B], F32, tag="t", name="zrow_ps")
            nc.tensor.transpose(zrow_ps[:], zp[:], ident[:B, :B])
            zrow = wp.tile([1, B], F32, tag="zrow")
            nc.scalar.copy(zrow[:], zrow_ps[:])
            z_in_d = dp.tile([1, B], F32, tag="z_in")
            z_out_d = dp.tile([NC, B], F32, tag="z_out")
            nc.sync.dma_start(out=z_in_d[:], in_=zrow[:])
            nc.gpsimd.collective_compute(
                "AllGather", mybir.AluOpType.bypass,
                replica_groups=[list(range(NC))],
                ins=[z_in_d[:].opt()], outs=[z_out_d[:].opt()],
            )
            zall = wp.tile([NC, B], F32, tag="zall")
            nc.sync.dma_start(out=zall[:], in_=z_out_d[:])
            zsum_ps = ps_t.tile([1, B], F32, tag="t", name="zsum_ps")
            nc.tensor.matmul(zsum_ps[:], ones8[:], zall[:], start=True, stop=True)
            zsum = wp.tile([1, B], F32, tag="zsum")
            nc.scalar.copy(zsum[:], zsum_ps[:])
            zcol_ps = ps_t.tile([B, 1], F32, tag="t", name="zcol_ps")
            nc.tensor.transpose(zcol_ps[:], zsum[:], ident[:1, :1])
            zcol = wp.tile([B, 1], F32, tag="zcol")
            nc.scalar.copy(zcol[:], zcol_ps[:])
            rz = wp.tile([B, 1], F32, tag="rz")
            nc.vector.reciprocal(rz[:], zcol[:])
            scl = wp.tile([B, 1], F32, tag="scl")
            nc.vector.tensor_mul(scl[:], pga, rz[:])

            # final = p_gen * softmax + scattered attn; fused per block
            for k in range(NBLK):
                blk_sl = slice(k * BLK, (k + 1) * BLK)
                acc = sc_tiles[0][k]
                for r in range(1, n_rounds):
                    nc.vector.tensor_add(acc[:], acc[:], sc_tiles[r][k][:])
                fb = encp.tile([B, BLK], F32, tag="fb", name=f"fb{k}")
                nc.vector.scalar_tensor_tensor(
                    out=fb[:], in0=expd[:, blk_sl], scalar=scl[:, :1],
                    in1=acc[:], op0=mybir.AluOpType.mult,
                    op1=mybir.AluOpType.add)
                nc.sync.dma_start(out=fin[:, blk_sl], in_=fb[:])

            _bsp_cm.__exit__(None, None, None)
            _xp_cm.__exit__(None, None, None)

    nc.compile()
    return nc


def _prep_inputs(inputs):
    """Host-side sharding / transposition. Returns (in_maps, n_rounds)."""
    emb = _f32(inputs["embedding"])
    y = _i32(np.asarray(inputs["y_t_1"]).reshape(B, 1))
    ct1T = _f32(np.asarray(inputs["c_t_1"]).T)                    # [512, 64]
    h0T = _f32(np.asarray(inputs["h0"]).T)                        # [256, 64]
    c0 = _f32(inputs["c0"])                                       # [64, 256]
    ef3 = _f32(inputs["encoder_feature"]).reshape(B, T, H2)
    enc3 = _f32(inputs["encoder_outputs"])                        # [64, 400, 512]
    stmt = _f32(inputs["stmt_feature"])
    mask = _f32(inputs["enc_padding_mask"])

    wxcT = _f32(np.asarray(inputs["Wxc"]).T)                      # [640, 128]
    bxc = _f32(np.asarray(inputs["bxc"]).reshape(E, 1))
    wihT = _f32(np.asarray(inputs["W_ih"]).T)                     # [128, 1024]
    whhT = _f32(np.asarray(inputs["W_hh"]).T)                     # [256, 1024]
    bgr = _f32((np.asarray(inputs["b_ih"]) +
                np.asarray(inputs["b_hh"])).reshape(1, 4 * H))
    wdpT = _bf16(np.asarray(inputs["Wdp"]).T)                     # [512, 512]
    bdpr = _bf16(np.asarray(inputs["bdp"]).reshape(1, H2))
    wvT = _bf16(np.asarray(inputs["Wv"]).T)                       # [512, 1]
    wpgT = _bf16(np.asarray(inputs["Wpg"]).T)                     # [1152, 1]
    bpg = _f32(np.full((BC, 1), np.asarray(inputs["bpg"]).reshape(-1)[0]))
    wo1T = _bf16(np.asarray(inputs["Wo1"]).T)                     # [768, 256]
    bo1r = _bf16(np.asarray(inputs["bo1"]).reshape(1, H))

    wo2 = _f32(inputs["Wo2"])                                     # [50000, 256]
    bo2 = _f32(inputs["bo2"])                                     # [50000]
    wo2T_pad = np.zeros((2 * E, VPAD), np.float32)
    wo2T_pad[:, :V] = wo2.T
    wo2T_pad = wo2T_pad.astype(NP_BF16)
    bo2_pad = np.full((VPAD,), NEG_BIG, np.float32)
    bo2_pad[:V] = bo2
    bo2_pad = bo2_pad.astype(NP_BF16)

    idx_maps = _plan_scatter(inputs["enc_batch_extend_vocab"])
    n_rounds = len(idx_maps)

    in_maps = []
    for c in range(NC):
        rb = slice(c * BC, (c + 1) * BC)
        vb = slice(c * VC, (c + 1) * VC)
        m = {
            "y_idx": y[rb],
            "emb": emb,
            "ct1T": np.ascontiguousarray(ct1T[:, rb]),
            "h0T": np.ascontiguousarray(h0T[:, rb]),
            "c0r": np.ascontiguousarray(c0[rb]),
            "efT": _bf16(ef3[rb].reshape(BC * T, H2).T),          # [512, 3200]
            "enc": _bf16(np.pad(enc3[rb], ((0, 0), (0, 512 - T), (0, 0))
                                ).reshape(BC * 512, H2)),
            "stmt": np.ascontiguousarray(stmt[rb]),
            "maskin": np.ascontiguousarray(mask[rb]),
            "wxcT": wxcT, "bxc": bxc, "wihT": wihT, "whhT": whhT,
            "bgr": bgr, "wdpT": wdpT, "bdpr": bdpr, "wvT": wvT,
            "wpgT": wpgT, "bpgs": bpg, "wo1T": wo1T, "bo1r": bo1r,
            "wo2T": np.ascontiguousarray(wo2T_pad[:, vb]),
            "bo2r": np.ascontiguousarray(bo2_pad[vb].reshape(1, VC)),
        }
        for r in range(n_rounds):
            m[f"scidx{r}"] = np.ascontiguousarray(idx_maps[r][c])
        in_maps.append(m)
    return in_maps, n_rounds


def run(inputs, trace=False):
    """Build + run; returns (outputs_tuple, BassKernelResults)."""
    in_maps, n_rounds = _prep_inputs(inputs)
    nc = build_program(n_rounds)
    res = run_bass_kernel_spmd(
        nc, in_maps, core_ids=list(range(NC)), trace=trace,
        trace_cores=[0] if trace else None,
    )
    rs = res.results

    final = np.concatenate([rs[c]["fin"] for c in range(NC)], axis=1)[:, :VFULL]
    h_s = np.concatenate([rs[c]["hs_o"] for c in range(NC)], axis=0)
    c_s = np.concatenate([rs[c]["cs_o"] for c in range(NC)], axis=0)
    c_t = np.concatenate([rs[c]["ct_o"] for c in range(NC)], axis=0)
    attn = np.concatenate([rs[c]["at_o"] for c in range(NC)], axis=0)
    p_gen = np.concatenate([rs[c]["pg_o"] for c in range(NC)], axis=0)
    coverage = _f32(inputs["coverage"])
    outs = (final, h_s, c_s, c_t, attn, p_gen, coverage)
    return outs, res


def kernel(**inputs):
    outs, _ = run(inputs, trace=False)
    return outs


# revision 17
# speedup vs baseline: 1.0874x; 1.0507x over previous
"""Trainium2 Bass kernel for the pointer-generator decoder step (nn_Decoder).

Strategy (8 NeuronCores):
  - Phase 1 (LSTM + attention): data-parallel over batch. Core c owns batch
    rows [8c, 8c+8). Encoder tensors (the big per-batch traffic) are sharded
    by batch and shipped in bf16.
  - Phase 2 (vocab projection + softmax + scatter): tensor-parallel over the
    vocab axis. Wo2/bo2 (bf16) and the final distribution are sharded into 8
    column blocks of 6400 (padded 50500 -> 51200).
  - Cross-core glue: one small AllGather of (output, p_gen, attn) after
    phase 1, one tiny AllGather of the partial softmax denominators.
  - The pointer-copy scatter-add uses gpsimd local_scatter: per-partition
    (per-batch-row) scatter of attn values along the vocab axis with
    host-built int16 position maps, in 4 column blocks of 1600 per round
    (duplicate targets go to later rounds), summed into the distribution
    with DVE adds. No indirect DMA involved.

Precision: bf16 for the traffic/compute-heavy matmul paths (tanh/scores
inputs, c_t contraction, vocab projection); f16 for the scattered attn
values; fp32 for the LSTM, softmax statistics, and everything written out.

Self-contained: hardcodes all shapes from the problem spec.
"""

import ml_dtypes
import numpy as np

import concourse.bacc as bacc
import concourse.bass as bass
import concourse.mybir as mybir
import concourse.tile as tile
from concourse.bass_utils import run_bass_kernel_spmd
from concourse.masks import make_identity

F32 = mybir.dt.float32
BF16 = mybir.dt.bfloat16
F16 = mybir.dt.float16
I16 = mybir.dt.int16
I32 = mybir.dt.int32
NP_BF16 = ml_dtypes.bfloat16

NC = 8                      # cores
B, T, H, E, V, X = 64, 400, 256, 128, 50000, 500
BC = B // NC                # batch rows per core = 8
VFULL = V + X               # 50500
VC = 6400                   # vocab columns per core (8*6400 = 51200 >= 50500)
VPAD = VC * NC
H2 = 2 * H                  # 512
NEG_BIG = -200.0            # pad bias -> exp() == 0 in f32
NBLK = 4                    # local_scatter column blocks per shard
BLK = VC // NBLK            # 1600 (< 2048 gpsimd local limit)

# vocab matmul column tiling
VT_SIZES = [512] * 12 + [256]          # sum = 6400
assert sum(VT_SIZES) == VC


def _f32(x):
    return np.ascontiguousarray(np.asarray(x), dtype=np.float32)


def _bf16(x):
    return np.ascontiguousarray(np.asarray(x, dtype=np.float32).astype(NP_BF16))


def _i32(x):
    return np.ascontiguousarray(np.asarray(x), dtype=np.int32)


def _plan_scatter(ebv: np.ndarray):
    """Host-side plan for the pointer scatter-add via gpsimd local_scatter.

    Returns idx_maps[r][c] = int16 [B, NBLK, T]: for round r, core c, block k:
    idx_maps[r][c][b, k, t] = local position (0..BLK) of target ebv[b, t]
    within block k of core c's shard if that pair belongs to (c, k, r),
    else -1. Within one (b, c, k, r) all positions are unique.
    """
    ebv = np.asarray(ebv).astype(np.int64).reshape(B, T)
    core = ebv // VC
    jl = ebv - core * VC
    blk = jl // BLK
    pos = jl - blk * BLK

    # occurrence rank of each (b, target) pair
    occ = np.zeros((B, T), np.int64)
    for b in range(B):
        seen = {}
        row = ebv[b]
        for t in range(T):
            v = int(row[t])
            occ[b, t] = seen.get(v, 0)
            seen[v] = occ[b, t] + 1
    R = int(occ.max()) + 1

    idx_maps = []
    for r in range(R):
        per_core = []
        for c in range(NC):
            m = np.full((B, NBLK, T), -1, np.int16)
            sel = (core == c) & (occ == r)
            bb, tt = np.nonzero(sel)
            m[bb, blk[bb, tt], tt] = pos[bb, tt].astype(np.int16)
            per_core.append(m)
        idx_maps.append(per_core)
    return idx_maps


def build_program(n_rounds):
    """Build the SPMD Bass program (same on all cores)."""
    nc = bacc.Bacc("TRN2", target_bir_lowering=False, debug=False, num_devices=NC)

    # ---------------- I/O declarations ----------------
    def din(name, shape, dtype=F32):
        return nc.dram_tensor(name, list(shape), dtype, kind="ExternalInput")

    def dout(name, shape, dtype=F32):
        return nc.dram_tensor(name, list(shape), dtype, kind="ExternalOutput")

    y_idx = din("y_idx", [BC, 1], I32)
    emb = din("emb", [V, E])
    ct1T = din("ct1T", [H2, BC])          # c_t_1 transposed slice
    h0T = din("h0T", [H, BC])
    c0r = din("c0r", [BC, H])             # c0 rows slice
    efT = din("efT", [H2, BC * T], BF16)  # encoder_feature transposed slice
    enc = din("enc", [BC * 512, H2], BF16)  # encoder_outputs slice (t padded)
    stmt = din("stmt", [BC, T])
    maskin = din("maskin", [BC, T])
    wxcT = din("wxcT", [H2 + E, E])       # [640, 128]
    bxc = din("bxc", [E, 1])
    wihT = din("wihT", [E, 4 * H])        # [128, 1024]
    whhT = din("whhT", [H, 4 * H])        # [256, 1024]
    bgr = din("bgr", [1, 4 * H])          # (b_ih + b_hh) row
    wdpT = din("wdpT", [H2, H2], BF16)    # [512, 512]
    bdpr = din("bdpr", [1, H2], BF16)
    wvT = din("wvT", [H2, 1], BF16)       # [512, 1]
    wpgT = din("wpgT", [4 * H + E, 1], BF16)  # [1152, 1]
    wo1T = din("wo1T", [3 * H, H], BF16)  # [768, 256]
    bo1r = din("bo1r", [1, H], BF16)
    wo2T = din("wo2T", [E * 2, VC], BF16)  # [256, 6400] shard
    bo2r = din("bo2r", [1, VC], BF16)     # padded with NEG_BIG
    bpgs = din("bpgs", [BC, 1])           # p_gen bias (replicated column)
    scidx = [din(f"scidx{r}", [B, NBLK, T], I16) for r in range(n_rounds)]

    fin = dout("fin", [B, VC])            # final_dist shard
    hs_o = dout("hs_o", [BC, H])
    cs_o = dout("cs_o", [BC, H])
    ct_o = dout("ct_o", [BC, H2])
    at_o = dout("at_o", [BC, T])
    pg_o = dout("pg_o", [BC, 1])

    AGW = 672                             # allgather row width (256+1+400 padded)

    Sig = mybir.ActivationFunctionType.Sigmoid
    Tanh = mybir.ActivationFunctionType.Tanh
    Exp = mybir.ActivationFunctionType.Exp
    Ident = mybir.ActivationFunctionType.Identity

    with tile.TileContext(nc) as tc:
        with (
            tc.tile_pool(name="const", bufs=1) as cp,
            tc.tile_pool(name="work", bufs=1) as wp,
            tc.tile_pool(name="encp", bufs=2) as encp,
            tc.tile_pool(name="ps_t", bufs=2, space="PSUM") as ps_t,
            tc.tile_pool(name="ps_mm", bufs=2, space="PSUM") as ps_mm,
            tc.tile_pool(name="ps_row", bufs=2, space="PSUM") as ps_row,
            tc.tile_pool(name="ps_lg", bufs=2, space="PSUM") as ps_lg,
            tc.tile_pool(name="dram", bufs=1, space="DRAM") as dp,
        ):
            # ---------------- constants / weights to SBUF ----------------
            ident = cp.tile([128, 128], F32)
            make_identity(nc, ident[:])
            identb = cp.tile([128, 128], BF16)
            make_identity(nc, identb[:])
            ones18 = cp.tile([1, 8], F32)
            nc.gpsimd.memset(ones18[:], 1.0)
            ones18b = cp.tile([1, 8], BF16)
            nc.gpsimd.memset(ones18b[:], 1.0)
            ones1 = cp.tile([1, 64], BF16)
            nc.gpsimd.memset(ones1[:], 1.0)
            ones8 = cp.tile([8, 1], F32)
            nc.gpsimd.memset(ones8[:], 1.0)

            def loadt(name, shape, src_ap, dtype=F32):
                t = cp.tile(shape, dtype, name=name)
                nc.sync.dma_start(out=t[:], in_=src_ap)
                return t

            # embedding gather first -- the x-projection chain starts with it
            yidx_t = loadt("yidx_t", [BC, 1], y_idx[:, :], dtype=I32)
            yemb = wp.tile([BC, E], F32, tag="yemb")
            nc.gpsimd.indirect_dma_start(
                out=yemb[:], out_offset=None, in_=emb[:, :],
                in_offset=bass.IndirectOffsetOnAxis(ap=yidx_t[:, :1], axis=0),
            )

            # big phase-2 weight on the ACT HWDGE ring so it streams in
            # parallel with the attention-phase loads on the SP ring
            wo2_t = cp.tile([128, 2, VC], BF16, name="wo2_t")
            nc.scalar.dma_start(out=wo2_t[:],
                                in_=wo2T[:, :].rearrange("(k p) v -> p k v", p=128))

            wxc_t = loadt("wxc_t", [128, 5, E],
                          wxcT[:, :].rearrange("(k p) m -> p k m", p=128))
            wih_t = loadt("wih_t", [128, 4 * H], wihT[:, :])
            whh_t = loadt("whh_t", [128, 2, 4 * H],
                          whhT[:, :].rearrange("(k p) m -> p k m", p=128))
            wdp_t = loadt("wdp_t", [128, 4, H2],
                          wdpT[:, :].rearrange("(k p) m -> p k m", p=128),
                          dtype=BF16)
            wv_t = loadt("wv_t", [128, 4, 1],
                         wvT[:, :].rearrange("(k p) m -> p k m", p=128), dtype=BF16)
            wpg_t = loadt("wpg_t", [128, 9, 1],
                          wpgT[:, :].rearrange("(k p) m -> p k m", p=128),
                          dtype=BF16)
            wo1_t = loadt("wo1_t", [128, 6, H],
                          wo1T[:, :].rearrange("(k p) m -> p k m", p=128),
                          dtype=BF16)
            bgr_t = loadt("bgr_t", [1, 4 * H], bgr[:, :])
            bdpr_t = loadt("bdpr_t", [1, H2], bdpr[:, :], dtype=BF16)
            bo1r_t = loadt("bo1r_t", [1, H], bo1r[:, :], dtype=BF16)
            bxc_t = loadt("bxc_t", [E, 1], bxc[:, :])
            bpg_t = loadt("bpg_t", [BC, 1], bpgs[:, :])

            ct1_t = loadt("ct1_t", [128, 4, BC],
                          ct1T[:, :].rearrange("(k p) b -> p k b", p=128))
            h0_t = loadt("h0_t", [128, 2, BC],
                         h0T[:, :].rearrange("(k p) b -> p k b", p=128))
            c0r_t = loadt("c0r_t", [BC, H], c0r[:, :])
            stmt_t = cp.tile([BC, T], F32, name="stmt_t")
            nc.scalar.dma_start(out=stmt_t[:], in_=stmt[:, :])
            mask_t = cp.tile([BC, T], F32, name="mask_t")
            nc.scalar.dma_start(out=mask_t[:], in_=maskin[:, :])

            def transpose_f32(name, src_ap, p_out, f_out):
                """[f_out, p_out] <- transpose of src_ap [p_out, f_out]."""
                pT = ps_t.tile([128, 128], F32, tag="t", name=f"{name}_ps")
                nc.tensor.transpose(pT[:f_out, :p_out], src_ap,
                                    ident[:p_out, :p_out])
                t = wp.tile([f_out, p_out], F32, name=name)
                nc.scalar.copy(t[:], pT[:f_out, :p_out])
                return t

            def transpose_bf(name, src_bf_ap, p_out, f_out):
                """bf16 [f_out, p_out] <- transpose of bf16 src [p_out, f_out]."""
                pT = ps_t.tile([128, 128], BF16, tag="t", name=f"{name}_ps")
                nc.tensor.transpose(pT[:f_out, :p_out], src_bf_ap,
                                    identb[:p_out, :p_out])
                t = wp.tile([f_out, p_out], BF16, name=name)
                nc.vector.tensor_copy(t[:], pT[:f_out, :p_out])
                return t

            # ---------------- x projection ----------------
            yembT = transpose_f32("yembT", yemb[:], BC, E)

            # xT = WxcT.T-chunks @ [ct1T; yembT] + bxc   -> [128, 8]
            x_ps = ps_mm.tile([E, BC], F32, tag="mm")
            for k in range(5):
                rhs = ct1_t[:, k, :] if k < 4 else yembT[:]
                nc.tensor.matmul(x_ps[:], wxc_t[:, k, :], rhs,
                                 start=(k == 0), stop=(k == 4))
            xT = wp.tile([E, BC], F32, tag="xT")
            nc.scalar.activation(xT[:], x_ps[:], Ident, bias=bxc_t[:, :1])

            # ---------------- LSTM gates (row layout [8, 1024]) ---------------
            # gates[b, :] = x @ Wih.T + h0 @ Whh.T + b; order i|f|g|o
            gate_rows = []
            for half in range(2):                      # [0,512) / [512,1024)
                sl = slice(half * 512, (half + 1) * 512)
                g_ps = ps_row.tile([BC, 512], F32, tag="row", name=f"g_ps{half}")
                nc.tensor.matmul(g_ps[:], ones18[:, :BC], bgr_t[:, sl],
                                 start=True, stop=False)
                nc.tensor.matmul(g_ps[:], xT[:], wih_t[:, sl],
                                 start=False, stop=False)
                for k in range(2):
                    nc.tensor.matmul(g_ps[:], h0_t[:, k, :], whh_t[:, k, sl],
                                     start=False, stop=(k == 1))
                gate_rows.append(g_ps)
            sig_if = wp.tile([BC, 512], F32, tag="sig_if")
            nc.scalar.activation(sig_if[:], gate_rows[0][:], Sig)
            tanh_g = wp.tile([BC, H], F32, tag="tanh_g")
            nc.scalar.activation(tanh_g[:], gate_rows[1][:, 0:H], Tanh)
            sig_o = wp.tile([BC, H], F32, tag="sig_o")
            nc.scalar.activation(sig_o[:], gate_rows[1][:, H:2 * H], Sig)

            m1 = wp.tile([BC, H], F32, tag="m1")
            nc.vector.tensor_mul(m1[:], sig_if[:, H:2 * H], c0r_t[:])
            m2 = wp.tile([BC, H], F32, tag="m2")
            nc.vector.tensor_mul(m2[:], sig_if[:, 0:H], tanh_g[:])
            cs_row = wp.tile([BC, H], F32, tag="cs_row")
            nc.vector.tensor_add(cs_row[:], m1[:], m2[:])
            tanh_cs = wp.tile([BC, H], F32, tag="tanh_cs")
            nc.scalar.activation(tanh_cs[:], cs_row[:], Tanh)
            hs_row = wp.tile([BC, H], F32, tag="hs_row")
            nc.vector.tensor_mul(hs_row[:], sig_o[:], tanh_cs[:])
            nc.sync.dma_start(out=hs_o[:, :], in_=hs_row[:])
            nc.sync.dma_start(out=cs_o[:, :], in_=cs_row[:])

            hs_bf = wp.tile([BC, H], BF16, tag="hs_bf")
            nc.vector.tensor_copy(hs_bf[:], hs_row[:])
            cs_bf = wp.tile([BC, H], BF16, tag="cs_bf")
            nc.vector.tensor_copy(cs_bf[:], cs_row[:])
            hsT = [transpose_bf(f"hsT{k}", hs_bf[:, k * 128:(k + 1) * 128],
                                BC, 128) for k in range(2)]
            csT = [transpose_bf(f"csT{k}", cs_bf[:, k * 128:(k + 1) * 128],
                                BC, 128) for k in range(2)]
            sthT = hsT + csT     # s_t_hat^T = [h_s; c_s] as 4 chunks of [128, 8]

            # ---------------- attention ----------------
            # dec_fea row [8, 512] then transpose to per-chunk bias columns
            d_ps = ps_row.tile([BC, H2], F32, tag="row", name="d_ps")
            nc.tensor.matmul(d_ps[:], ones18b[:, :BC], bdpr_t[:, :],
                             start=True, stop=False)
            for k in range(4):
                nc.tensor.matmul(d_ps[:], sthT[k][:], wdp_t[:, k, :],
                                 start=False, stop=(k == 3))
            dec_row = wp.tile([BC, H2], BF16, tag="dec_row")
            nc.vector.tensor_copy(dec_row[:], d_ps[:])
            decT = [transpose_bf(f"decT{k}", dec_row[:, k * 128:(k + 1) * 128],
                                 BC, 128) for k in range(4)]

            # scores[b, t] accumulated over 4 n-chunks; per-b PSUM rows.
            # Engine APs must start at partition 0/32/64/96, so per-b rows are
            # written into a [1, 8*T] free-concat tile and reshaped via DRAM.
            esc_all = wp.tile([1, BC * T], F32, tag="esc_all")
            with tc.tile_pool(name="eft", bufs=2) as efp, \
                 tc.tile_pool(name="th", bufs=4) as thp:
                th_ts = []
                for nci in range(4):
                    ef_t = efp.tile([128, BC * T], BF16, tag="ef", name=f"ef{nci}")
                    nc.sync.dma_start(out=ef_t[:],
                                      in_=efT[nci * 128:(nci + 1) * 128, :])
                    # ef + dec_fea (free-broadcast per b), then one big tanh
                    ta = thp.tile([128, BC, T], BF16, tag="th", name=f"ta{nci}")
                    nc.vector.tensor_add(
                        ta[:],
                        ef_t[:].rearrange("p (b t) -> p b t", b=BC),
                        decT[nci][:].unsqueeze(2).to_broadcast([128, BC, T]))
                    th = ta[:].rearrange("p b t -> p (b t)")
                    nc.scalar.activation(th, th, Tanh)
                    th_ts.append(th)
                for b in range(BC):
                    sc_ps = ps_row.tile([1, T], F32, tag="row", name=f"sc{b}")
                    for nci in range(4):
                        nc.tensor.matmul(sc_ps[:, :], wv_t[:, nci, :],
                                         th_ts[nci][:, b * T:(b + 1) * T],
                                         start=(nci == 0), stop=(nci == 3))
                    # exp while still in PSUM; write segment b of esc_all
                    nc.scalar.activation(esc_all[:, b * T:(b + 1) * T],
                                         sc_ps[:, :], Exp)
            # reshape [1, B*T] -> [B, T] via DRAM bounce (SBUF->SBUF
            # partition-crossing reshape DMAs are not HW-reliable)
            esc_d = dp.tile([BC, T], F32, tag="esc_d", name="esc_d")
            nc.sync.dma_start(out=esc_d[:].flatten().unsqueeze(0), in_=esc_all[:1, :])
            esc = wp.tile([BC, T], F32, tag="esc")
            nc.sync.dma_start(out=esc[:, :], in_=esc_d[:])

            # softmax over t (no max-subtraction needed: |scores| < ~8)
            nc.vector.tensor_mul(esc[:], esc[:], mask_t[:])
            z1 = wp.tile([BC, 1], F32, tag="z1")
            nc.vector.tensor_reduce(z1[:], esc[:], mybir.AxisListType.X,
                                    mybir.AluOpType.add)
            rz1 = wp.tile([BC, 1], F32, tag="rz1")
            nc.vector.reciprocal(rz1[:], z1[:])
            sm = wp.tile([BC, T], F32, tag="sm")
            nc.vector.tensor_mul(sm[:], stmt_t[:], mask_t[:])
            attn = wp.tile([BC, T], F32, tag="attn")
            nc.vector.scalar_tensor_tensor(
                out=attn[:], in0=esc[:], scalar=rz1[:, :1], in1=sm[:],
                op0=mybir.AluOpType.mult, op1=mybir.AluOpType.add)
            nc.sync.dma_start(out=at_o[:, :], in_=attn[:])

            # attn^T chunks (bf16, zero-padded to 512) for the c_t matmul
            attn_bf = wp.tile([BC, 512], BF16, tag="attn_bf")
            nc.gpsimd.memset(attn_bf[:], 0.0)
            nc.vector.tensor_copy(attn_bf[:, :T], attn[:])
            attnT = []
            for tch in range(4):
                aT = transpose_bf(f"attnT{tch}",
                                  attn_bf[:, tch * 128:(tch + 1) * 128], BC, 128)
                attnT.append(aT)

            # c_t[b, :] = sum_t attn[b, t] * enc[b, t, :]
            ct_all = wp.tile([1, BC * H2], F32, tag="ct_all")
            for b in range(BC):
                e4 = encp.tile([128, 4, H2], BF16, tag="enc4", name=f"e4_{b}")
                nc.scalar.dma_start(
                    out=e4[:],
                    in_=enc[b * 512:(b + 1) * 512, :].rearrange(
                        "(a p) f -> p a f", p=128),
                )
                ct_ps = ps_row.tile([1, H2], F32, tag="row", name=f"ct{b}")
                for tch in range(4):
                    nc.tensor.matmul(ct_ps[:, :], attnT[tch][:, b:b + 1],
                                     e4[:, tch, :],
                                     start=(tch == 0), stop=(tch == 3))
                nc.vector.tensor_copy(ct_all[:, b * H2:(b + 1) * H2], ct_ps[:, :])
            ct_d = dp.tile([BC, H2], F32, tag="ct_d", name="ct_d")
            nc.sync.dma_start(out=ct_d[:].flatten().unsqueeze(0), in_=ct_all[:1, :])
            ct_row = wp.tile([BC, H2], F32, tag="ct_row")
            nc.sync.dma_start(out=ct_row[:, :], in_=ct_d[:])
            nc.sync.dma_start(out=ct_o[:, :], in_=ct_row[:])

            ct_bf = wp.tile([BC, H2], BF16, tag="ct_bf")
            nc.vector.tensor_copy(ct_bf[:], ct_row[:])
            ctT = [transpose_bf(f"ctT{k}", ct_bf[:, k * 128:(k + 1) * 128],
                                BC, 128) for k in range(4)]
            xT_bf = wp.tile([E, BC], BF16, tag="xT_bf")
            nc.vector.tensor_copy(xT_bf[:], xT[:])

            # ---------------- p_gen (row layout -> [8, 1] directly) -----------
            pg_ps = ps_mm.tile([BC, 1], F32, tag="mm", name="pg_ps")
            pg_chunks = ctT + sthT + [xT_bf]
            for k in range(9):
                nc.tensor.matmul(pg_ps[:], pg_chunks[k][:], wpg_t[:, k, :],
                                 start=(k == 0), stop=(k == 8))
            pg_col = wp.tile([BC, 1], F32, tag="pg_col")
            nc.scalar.activation(pg_col[:], pg_ps[:], Sig, bias=bpg_t[:, :1])
            nc.sync.dma_start(out=pg_o[:, :], in_=pg_col[:])

            # ---------------- output projection (row layout [8, 256]) ---------
            out_chunks = hsT + ctT        # [h_s; c_t] -> 6 chunks of 128
            o_ps = ps_row.tile([BC, H], F32, tag="row", name="o_ps")
            nc.tensor.matmul(o_ps[:], ones18b[:, :BC], bo1r_t[:, :],
                             start=True, stop=False)
            for k in range(6):
                nc.tensor.matmul(o_ps[:], out_chunks[k][:], wo1_t[:, k, :],
                                 start=False, stop=(k == 5))
            out_row = wp.tile([BC, H], F32, tag="out_row")
            nc.scalar.copy(out_row[:], o_ps[:])

            # ---------------- AllGather #1 ----------------
            agin = wp.tile([BC, AGW], F32, tag="agin")
            nc.gpsimd.memset(agin[:], 0.0)
            nc.vector.tensor_copy(agin[:, 0:H], out_row[:])
            nc.vector.tensor_copy(agin[:, H:H + 1], pg_col[:])
            nc.vector.tensor_copy(agin[:, H + 1:H + 1 + T], attn[:])
            ag_in_d = dp.tile([BC, AGW], F32, tag="ag_in")
            ag_out_d = dp.tile([B, AGW], F32, tag="ag_out")
            nc.sync.dma_start(out=ag_in_d[:], in_=agin[:])
            nc.gpsimd.collective_compute(
                "AllGather", mybir.AluOpType.bypass,
                replica_groups=[list(range(NC))],
                ins=[ag_in_d[:].opt()], outs=[ag_out_d[:].opt()],
            )
            aga = wp.tile([B, AGW], F32, tag="aga")
            nc.sync.dma_start(out=aga[:], in_=ag_out_d[:, :])
            oa = aga[:, 0:H]
            pga = aga[:, H:H + 1]
            ata = aga[:, H + 1:H + 1 + T]

            # ---------------- pointer scatter (gpsimd local_scatter) ----------
            om = wp.tile([B, 1], F32, tag="om")      # 1 - p_gen
            nc.vector.tensor_scalar(om[:], pga, -1.0, 1.0,
                                    mybir.AluOpType.mult, mybir.AluOpType.add)
            axp16 = wp.tile([B, T], BF16, tag="axp16")
            nc.vector.tensor_mul(axp16[:], ata, om[:].to_broadcast([B, T]))

            sc_tiles = [[None] * NBLK for _ in range(n_rounds)]
            for r in range(n_rounds):
                idx_t = wp.tile([B, NBLK, T], I16, tag=f"scidx{r}")
                nc.sync.dma_start(out=idx_t[:], in_=scidx[r][:, :, :])
                for k in range(NBLK):
                    st = wp.tile([B, BLK], BF16, tag=f"sct{r}_{k}")
                    nc.gpsimd.local_scatter(
                        out_ap=st[:], data_ap=axp16[:], idxs_ap=idx_t[:, k, :],
                        channels=B, num_elems=BLK, num_idxs=T,
                    )
                    sc_tiles[r][k] = st

            # ---------------- logits + exp (vocab shard) ----------------
            _xp_cm = tc.tile_pool(name="expp", bufs=1)
            xp = _xp_cm.__enter__()
            _bsp_cm = tc.tile_pool(name="bo2p", bufs=2)
            bsp = _bsp_cm.__enter__()

            oa_bf = wp.tile([B, H], BF16, tag="oa_bf")
            nc.vector.tensor_copy(oa_bf[:], oa)
            oaT = []
            for k in range(2):
                pT = ps_t.tile([128, B], BF16, tag="t", name=f"oaT{k}_ps")
                nc.tensor.transpose(pT[:], oa_bf[:, k * 128:(k + 1) * 128],
                                    identb[:B, :B])
                ot = wp.tile([128, B], BF16, tag=f"oaT{k}")
                nc.vector.tensor_copy(ot[:], pT[:])
                oaT.append(ot)

            expd = xp.tile([B, VC], BF16, tag="expd")
            zcols = wp.tile([B, len(VT_SIZES)], F32, tag="zcols")
            off = 0
            for vt, nv in enumerate(VT_SIZES):
                l_ps = ps_lg.tile([B, 512], F32, tag="lg", name=f"lg{vt}")
                sl = slice(off, off + nv)
                bo2s = bsp.tile([1, 512], BF16, tag="bo2s", name=f"bo2s{vt}")
                nc.scalar.dma_start(out=bo2s[:, :nv], in_=bo2r[:, sl])
                nc.tensor.matmul(l_ps[:, :nv], ones1[:, :B], bo2s[:, :nv],
                                 start=True, stop=False)
                for k in range(2):
                    nc.tensor.matmul(l_ps[:, :nv], oaT[k][:], wo2_t[:, k, sl],
                                     start=False, stop=(k == 1))
                nc.scalar.activation(expd[:, sl], l_ps[:, :nv], Exp,
                                     accum_out=zcols[:, vt:vt + 1])
                off += nv

            zp = wp.tile([B, 1], F32, tag="zp")
            nc.vector.tensor_reduce(zp[:], zcols[:], mybir.AxisListType.X,
                                    mybir.AluOpType.add)

            # ---------------- AllGather #2 (partial denominators) -------------
            zrow_ps = ps_t.tile([1, B], F32, tag="t", name="zrow_ps")
            nc.tensor.transpose(zrow_ps[:], zp[:], ident[:B, :B])
            zrow = wp.tile([1, B], F32, tag="zrow")
            nc.scalar.copy(zrow[:], zrow_ps[:])
            z_in_d = dp.tile([1, B], F32, tag="z_in")
            z_out_d = dp.tile([NC, B], F32, tag="z_out")
            nc.sync.dma_start(out=z_in_d[:], in_=zrow[:])
            nc.gpsimd.collective_compute(
                "AllGather", mybir.AluOpType.bypass,
                replica_groups=[list(range(NC))],
                ins=[z_in_d[:].opt()], outs=[z_out_d[:].opt()],
            )
            zall = wp.tile([NC, B], F32, tag="zall")
            nc.sync.dma_start(out=zall[:], in_=z_out_d[:])
            zsum_ps = ps_t.tile([1, B], F32, tag="t", name="zsum_ps")
            nc.tensor.matmul(zsum_ps[:], ones8[:], zall[:], start=True, stop=True)
            zsum = wp.tile([1, B], F32, tag="zsum")
            nc.scalar.copy(zsum[:], zsum_ps[:])
            zcol_ps = ps_t.tile([B, 1], F32, tag="t", name="zcol_ps")
            nc.tensor.transpose(zcol_ps[:], zsum[:], ident[:1, :1])
            zcol = wp.tile([B, 1], F32, tag="zcol")
            nc.scalar.copy(zcol[:], zcol_ps[:])
            rz = wp.tile([B, 1], F32, tag="rz")
            nc.vector.reciprocal(rz[:], zcol[:])
            scl = wp.tile([B, 1], F32, tag="scl")
            nc.vector.tensor_mul(scl[:], pga, rz[:])

            # final = p_gen * softmax + scattered attn; fused per block
            for k in range(NBLK):
                blk_sl = slice(k * BLK, (k + 1) * BLK)
                acc = sc_tiles[0][k]
                for r in range(1, n_rounds):
                    nc.vector.tensor_add(acc[:], acc[:], sc_tiles[r][k][:])
                fb = encp.tile([B, BLK], F32, tag="fb", name=f"fb{k}")
                nc.vector.scalar_tensor_tensor(
                    out=fb[:], in0=expd[:, blk_sl], scalar=scl[:, :1],
                    in1=acc[:], op0=mybir.AluOpType.mult,
                    op1=mybir.AluOpType.add)
                nc.sync.dma_start(out=fin[:, blk_sl], in_=fb[:])

            _bsp_cm.__exit__(None, None, None)
            _xp_cm.__exit__(None, None, None)

    nc.compile()
    return nc


def _prep_inputs(inputs):
    """Host-side sharding / transposition. Returns (in_maps, n_rounds)."""
    emb = _f32(inputs["embedding"])
    y = _i32(np.asarray(inputs["y_t_1"]).reshape(B, 1))
    ct1T = _f32(np.asarray(inputs["c_t_1"]).T)                    # [512, 64]
    h0T = _f32(np.asarray(inputs["h0"]).T)                        # [256, 64]
    c0 = _f32(inputs["c0"])                                       # [64, 256]
    ef3 = _f32(inputs["encoder_feature"]).reshape(B, T, H2)
    enc3 = _f32(inputs["encoder_outputs"])                        # [64, 400, 512]
    stmt = _f32(inputs["stmt_feature"])
    mask = _f32(inputs["enc_padding_mask"])

    wxcT = _f32(np.asarray(inputs["Wxc"]).T)                      # [640, 128]
    bxc = _f32(np.asarray(inputs["bxc"]).reshape(E, 1))
    wihT = _f32(np.asarray(inputs["W_ih"]).T)                     # [128, 1024]
    whhT = _f32(np.asarray(inputs["W_hh"]).T)                     # [256, 1024]
    bgr = _f32((np.asarray(inputs["b_ih"]) +
                np.asarray(inputs["b_hh"])).reshape(1, 4 * H))
    wdpT = _bf16(np.asarray(inputs["Wdp"]).T)                     # [512, 512]
    bdpr = _bf16(np.asarray(inputs["bdp"]).reshape(1, H2))
    wvT = _bf16(np.asarray(inputs["Wv"]).T)                       # [512, 1]
    wpgT = _bf16(np.asarray(inputs["Wpg"]).T)                     # [1152, 1]
    bpg = _f32(np.full((BC, 1), np.asarray(inputs["bpg"]).reshape(-1)[0]))
    wo1T = _bf16(np.asarray(inputs["Wo1"]).T)                     # [768, 256]
    bo1r = _bf16(np.asarray(inputs["bo1"]).reshape(1, H))

    wo2 = _f32(inputs["Wo2"])                                     # [50000, 256]
    bo2 = _f32(inputs["bo2"])                                     # [50000]
    wo2T_pad = np.zeros((2 * E, VPAD), np.float32)
    wo2T_pad[:, :V] = wo2.T
    wo2T_pad = wo2T_pad.astype(NP_BF16)
    bo2_pad = np.full((VPAD,), NEG_BIG, np.float32)
    bo2_pad[:V] = bo2
    bo2_pad = bo2_pad.astype(NP_BF16)

    idx_maps = _plan_scatter(inputs["enc_batch_extend_vocab"])
    n_rounds = len(idx_maps)

    in_maps = []
    for c in range(NC):
        rb = slice(c * BC, (c + 1) * BC)
        vb = slice(c * VC, (c + 1) * VC)
        m = {
            "y_idx": y[rb],
            "emb": emb,
            "ct1T": np.ascontiguousarray(ct1T[:, rb]),
            "h0T": np.ascontiguousarray(h0T[:, rb]),
            "c0r": np.ascontiguousarray(c0[rb]),
            "efT": _bf16(ef3[rb].reshape(BC * T, H2).T),          # [512, 3200]
            "enc": _bf16(np.pad(enc3[rb], ((0, 0), (0, 512 - T), (0, 0))
                                ).reshape(BC * 512, H2)),
            "stmt": np.ascontiguousarray(stmt[rb]),
            "maskin": np.ascontiguousarray(mask[rb]),
            "wxcT": wxcT, "bxc": bxc, "wihT": wihT, "whhT": whhT,
            "bgr": bgr, "wdpT": wdpT, "bdpr": bdpr, "wvT": wvT,
            "wpgT": wpgT, "bpgs": bpg, "wo1T": wo1T, "bo1r": bo1r,
            "wo2T": np.ascontiguousarray(wo2T_pad[:, vb]),
            "bo2r": np.ascontiguousarray(bo2_pad[vb].reshape(1, VC)),
        }
        for r in range(n_rounds):
            m[f"scidx{r}"] = np.ascontiguousarray(idx_maps[r][c])
        in_maps.append(m)
    return in_maps, n_rounds


def run(inputs, trace=False):
    """Build + run; returns (outputs_tuple, BassKernelResults)."""
    in_maps, n_rounds = _prep_inputs(inputs)
    nc = build_program(n_rounds)
    res = run_bass_kernel_spmd(
        nc, in_maps, core_ids=list(range(NC)), trace=trace,
        trace_cores=[0] if trace else None,
    )
    rs = res.results

    final = np.concatenate([rs[c]["fin"] for c in range(NC)], axis=1)[:, :VFULL]
    h_s = np.concatenate([rs[c]["hs_o"] for c in range(NC)], axis=0)
    c_s = np.concatenate([rs[c]["cs_o"] for c in range(NC)], axis=0)
    c_t = np.concatenate([rs[c]["ct_o"] for c in range(NC)], axis=0)
    attn = np.concatenate([rs[c]["at_o"] for c in range(NC)], axis=0)
    p_gen = np.concatenate([rs[c]["pg_o"] for c in range(NC)], axis=0)
    coverage = _f32(inputs["coverage"])
    outs = (final, h_s, c_s, c_t, attn, p_gen, coverage)
    return outs, res


def kernel(**inputs):
    outs, _ = run(inputs, trace=False)
    return outs
